# revision 50
# baseline (speedup 1.0000x reference)
"""GPS (GraphGPS) forward pass on 8 Trainium2 NeuronCores.

Model (from the reference): 2 layers of
  SAGEConv(mean aggr) + residual + BN  ||  per-graph dense MHA + residual + BN
  -> sum branches -> MLP residual -> BN -> outer BN + relu + residual
then per-graph mean pool + linear head.

Sharding: one graph (1024 nodes) per core. The SAGE neighbor aggregation is
computed ReduceScatter-style: each core multiplies its LOCAL node features
h_c [1024, 256] against its src-slice of the dense edge-count matrix
A_c [1024 src x 8192 dst] (fp8 counts, exact small ints), producing partial
aggregates for ALL destinations; a ReduceScatter(add) then hands every core
the summed aggregate rows for its own 1024 destinations, which are scaled by
1/deg locally. This needs no AllGather of features at all. BatchNorm batch
stats are exchanged with small AllGathers (cheaper than AllReduce here) and
summed locally.

Device layout: features kept transposed (hT = [256 dims x 1024 nodes], dims
on partitions) so BN stats/apply are per-partition ops; h natural
([node, dim], from 16 PE transposes per layer) feeds the SAGE matmul as lhsT.
Attention: scores^T [keys, q] per (head, key-tile); exp on ACT; PV contracts
over keys with the 33-wide (V ++ ones) natural V so output lands natural
[q, d] with the softmax denominator on the same partition as its query row
(per-partition normalize), then 16 PE transposes take O back to d-major for
the out-projection.
"""
import numpy as np
import ml_dtypes

import concourse.bass as bass
import concourse.mybir as mybir
import concourse.tile as tile
from concourse.bass_utils import run_bass_kernel_spmd
from concourse.vector_clock import ScopedClock
from concourse.masks import make_identity

# ---------------------------------------------------------------------------
# Walrus workaround: this toolchain rejects >1 sync-wait command per
# instruction. Hoist excess waits onto same-engine NoOps / extra drains.
# ---------------------------------------------------------------------------
_MAX_WAITS = 1


def _split_waits_in_ordered(nc, ordered):
    for bb_name, insts in ordered.items():
        new_list = []
        for inst in insts:
            si = getattr(inst, "sync_info", None)
            if si is not None and si.on_wait and len(si.on_wait) > _MAX_WAITS:
                waits = list(si.on_wait)
                keep = waits[-_MAX_WAITS:]
                for w in waits[:-_MAX_WAITS]:
                    nop = mybir.InstNoOp(
                        name=nc.get_next_instruction_name(),
                        engine=inst.engine,
                        ins=[],
                        outs=[],
                        sync_info=mybir.SyncInfo(on_wait=[w], on_update=[]),
                    )
                    nop.debug = inst.debug
                    new_list.append(nop)
                si.on_wait[:] = keep
            new_list.append(inst)
        insts[:] = new_list


_orig_lower = tile.TileContext._lower_ordered_insts


def _patched_lower_ordered_insts(self, ordered):
    _split_waits_in_ordered(self.nc, ordered)
    return _orig_lower(self, ordered)


def _patched_drain_and_barrier(self, tick_clock, wait_clock):
    drain_inst = self.nc.sync.drain()
    wait_clock.add_sem_waits(drain_inst.ins, ScopedClock({None: tick_clock.global_clock}))
    si = drain_inst.ins.sync_info
    waits = list(si.on_wait) if si is not None else []
    if len(waits) > _MAX_WAITS:
        si.on_wait[:] = waits[:_MAX_WAITS]
        for w in waits[_MAX_WAITS:]:
            d2 = self.nc.sync.drain()
            d2.ins.sync_info = mybir.SyncInfo(on_wait=[w], on_update=[])
    self.nc.all_engine_barrier()
    assert self.sems is not None
    popped = self.nc._tile_sem_poison_stack.pop()
    assert popped is self._sem_poison
    self.nc.clear_and_free_semaphores(list(self.sems.allocated().values()))
    self.nc.all_engine_barrier()


tile.TileContext._lower_ordered_insts = _patched_lower_ordered_insts
tile.TileContext._drain_and_barrier = _patched_drain_and_barrier

# ---------------------------------------------------------------------------
# Problem constants (hardcoded per the task contract)
# ---------------------------------------------------------------------------
N, B, NPG = 8192, 8, 1024
D, H, DH, L = 256, 8, 32, 2
IN_C, OUT_D, E, DFF = 128, 64, 262144, 512
EPS = 1e-5
NCORES = 8
P = 128          # SBUF partitions
DT2 = D // P     # 2 dim tiles of 128
FT4 = DFF // P   # 4 ff tiles
NT8 = NPG // P   # 8 local node tiles
CH = 16          # dst chunks for the SAGE partial matmul
CHW = N // CH    # 512 dst per chunk
F32 = mybir.dt.float32
BF16 = mybir.dt.bfloat16
FP8 = mybir.dt.float8e4
AF = mybir.ActivationFunctionType
ALU = mybir.AluOpType
RG = [list(range(NCORES))]


def build_kernel():
    nc = bass.Bass()

    # ---- I/O declarations ----
    xT_in = nc.dram_tensor("xT", [P, NPG], BF16, kind="ExternalInput")
    at_in = nc.dram_tensor("at", [NPG, N], FP8, kind="ExternalInput")
    invd_in = nc.dram_tensor("invd", [1, N], BF16, kind="ExternalInput")
    # per-layer weights, host-transposed; leading dims packed for [128, ...] SBUF tiles
    wlT_in = nc.dram_tensor("wlT", [L, DT2, P, D], BF16, kind="ExternalInput")
    wrT_in = nc.dram_tensor("wrT", [L, DT2, P, D], BF16, kind="ExternalInput")
    wqT_in = nc.dram_tensor("wqT", [L, DT2, P, D], BF16, kind="ExternalInput")
    wkT_in = nc.dram_tensor("wkT", [L, DT2, P, D], BF16, kind="ExternalInput")
    wvT_in = nc.dram_tensor("wvT", [L, DT2, P, D], BF16, kind="ExternalInput")
    owT_in = nc.dram_tensor("owT", [L, DT2, P, D], BF16, kind="ExternalInput")
    w1T_in = nc.dram_tensor("w1T", [L, DT2, P, DFF], BF16, kind="ExternalInput")
    w2T_in = nc.dram_tensor("w2T", [L, FT4, P, D], BF16, kind="ExternalInput")
    w_inT_in = nc.dram_tensor("w_inT", [IN_C, D], BF16, kind="ExternalInput")
    w_outT_in = nc.dram_tensor("w_outT", [DT2, P, OUT_D], BF16, kind="ExternalInput")
    # biases / norm params, fp32; [idx, dt, p] so device holds [p, idx, dt]
    bias_in = nc.dram_tensor("biasv", [L, 8, DT2, P], F32, kind="ExternalInput")
    #   biasv[l]: 0=sage_b 1=qb 2=kb 3=ob 4=b2 5=b_in(l0) 6,7 spare
    b1_in = nc.dram_tensor("b1v", [L, FT4, P], F32, kind="ExternalInput")
    nrm_in = nc.dram_tensor("nrmv", [L, 8, DT2, P], F32, kind="ExternalInput")
    #   nrmv[l]: 0=n1_w 1=n1_b 2=n2_w 3=n2_b 4=n3_w 5=n3_b 6=bn_w 7=bn_b
    vb_in = nc.dram_tensor("vbr", [L, 1, D], BF16, kind="ExternalInput")
    bout_in = nc.dram_tensor("boutv", [OUT_D, 1], F32, kind="ExternalInput")

    y_out = nc.dram_tensor("y", [OUT_D, 1], F32, kind="ExternalOutput")

    with tile.TileContext(nc) as tc:
        with (
            tc.tile_pool(name="wpool", bufs=1) as wpool,      # persistent weights
            tc.tile_pool(name="featp", bufs=2) as featp,      # hT (old/new rotate)
            tc.tile_pool(name="natp", bufs=1) as natp,        # h natural + agg
            tc.tile_pool(name="qkp", bufs=1) as qkp,          # Q/K/V per layer
            tc.tile_pool(name="expp", bufs=16) as expp,       # exp(score) tiles
            tc.tile_pool(name="onp", bufs=1) as onp,          # O_nat / OT
            tc.tile_pool(name="xp", bufs=1) as xp,            # x1/x2/out/out2
            tc.tile_pool(name="stg", bufs=2) as stg,          # RS staging chunks
            tc.tile_pool(name="small", bufs=4) as small,      # stats etc
            tc.tile_pool(name="atp", bufs=4) as atp,          # A chunk stream
            tc.tile_pool(name="psA", bufs=2, space="PSUM") as psA,   # 2 banks
            tc.tile_pool(name="psS", bufs=2, space="PSUM") as psS,   # 4 banks
            tc.tile_pool(name="psV", bufs=1, space="PSUM") as psV,   # 1 bank
            tc.tile_pool(name="dram", bufs=2, space="DRAM") as dram,
        ):
            assert nc.vector.BN_STATS_FMAX >= 512

            # ---------------- load weights ----------------
            def load_w(shape, src_ap, name, dtype=BF16, pool=wpool):
                t = pool.tile(shape, dtype, tag=name, name=name)
                nc.sync.dma_start(out=t[:], in_=src_ap)
                return t

            # order matters: in_proj inputs + small params first so the first
            # matmuls aren't queued behind the big weight streams
            xT = load_w([P, NPG], xT_in[:, :], "xTw")
            w_inT = load_w([IN_C, D], w_inT_in[:, :], "w_inTw")
            biasv = [load_w([P, 8, DT2], bias_in[l].rearrange("i t p -> p i t"),
                            f"biasw{l}", F32) for l in range(L)]
            nrmv = [load_w([P, 8, DT2], nrm_in[l].rearrange("i t p -> p i t"),
                           f"nrmw{l}", F32) for l in range(L)]
            b1v = [load_w([P, FT4], b1_in[l].rearrange("t p -> p t"),
                          f"b1w{l}", F32) for l in range(L)]
            vbr = [load_w([1, D], vb_in[l], f"vbrw{l}") for l in range(L)]
            boutv = load_w([OUT_D, 1], bout_in[:, :], "boutw", F32)

            def load_packed(src, free, nm):
                # src [L, K, P, free] -> per-layer tiles [P, K, free]
                return [load_w([P, src.shape[1], free],
                               src[l].rearrange("k p f -> p k f"), f"{nm}{l}")
                        for l in range(L)]

            wqT = load_packed(wqT_in, D, "wqTw")
            wkT = load_packed(wkT_in, D, "wkTw")
            wvT = load_packed(wvT_in, D, "wvTw")
            owT = load_packed(owT_in, D, "owTw")
            wlT = load_packed(wlT_in, D, "wlTw")
            wrT = load_packed(wrT_in, D, "wrTw")
            w1T = load_packed(w1T_in, DFF, "w1Tw")
            w2T = load_packed(w2T_in, D, "w2Tw")
            w_outT = load_w([P, DT2, OUT_D], w_outT_in[:].rearrange("t p o -> p t o"),
                            "w_outTw")

            # global inv_deg, broadcast to all partitions: folded into the
            # SAGE partial drains (pre-ReduceScatter), so the RS result is
            # the finished mean aggregation
            invd_bc = wpool.tile([P, N], BF16, tag="invdbc", name="invdbc")
            iv_ap = invd_in[:, :]
            nc.sync.dma_start(
                out=invd_bc[:],
                in_=bass.AP(tensor=iv_ap.tensor, offset=iv_ap.offset,
                            ap=[[0, P]] + list(iv_ap.ap[1:])),
            )

            ones_row = wpool.tile([1, P], BF16)
            nc.vector.memset(ones_row[:], 1.0)
            eps_t = wpool.tile([P, 1], F32)
            nc.vector.memset(eps_t[:], EPS)
            ident = wpool.tile([P, P], F32)
            make_identity(nc, ident[:])

            def bias_ap(l, idx, dt):
                return biasv[l][:, idx, dt:dt + 1]

            def nrm_ap(l, idx, dt):
                return nrmv[l][:, idx, dt:dt + 1]

            # generic matmul into psA 512-slices with per-slice drain callback
            def mm_slices(lhsT_aps, rhs_aps, nfree, drain, slice_w=512):
                for s0 in range(0, nfree, slice_w):
                    w = min(slice_w, nfree - s0)
                    ps = psA.tile([P, 512], F32, space="PSUM", tag="a", name="a")
                    nk = len(lhsT_aps)
                    for k in range(nk):
                        nc.tensor.matmul(
                            out=ps[:, 0:w], lhsT=lhsT_aps[k],
                            rhs=rhs_aps[k][:, s0:s0 + w],
                            start=(k == 0), stop=(k == nk - 1),
                        )
                    drain(ps, s0, w)

            # ---------------- in_proj ----------------
            hT_f = [featp.tile([P, NPG], F32, tag=f"hTf{dt}", name=f"hTf{dt}")
                    for dt in range(DT2)]
            hT_b = [featp.tile([P, NPG], BF16, tag=f"hTb{dt}", name=f"hTb{dt}")
                    for dt in range(DT2)]
            for dt in range(DT2):
                def drain_in(ps, s0, w, dt=dt):
                    nc.scalar.activation(out=hT_f[dt][:, s0:s0 + w], in_=ps[:, 0:w],
                                         func=AF.Identity, bias=bias_ap(0, 5, dt))
                mm_slices([w_inT[:, dt * P:(dt + 1) * P]], [xT[:]], NPG, drain_in)
                nc.gpsimd.tensor_copy(out=hT_b[dt][:], in_=hT_f[dt][:])

            # ---------------- layers ----------------
            for l in range(L):
                # ---- A chunk prefetch (first 4; rest issued inside interleave)
                at_tiles = [None] * CH

                def fetch_chunk(c):
                    t = atp.tile([P, NT8, CHW], FP8, tag="att", name="att")
                    nc.sync.dma_start(
                        out=t[:],
                        in_=at_in[:, c * CHW:(c + 1) * CHW]
                        .rearrange("(kt p) f -> p kt f", p=P))
                    at_tiles[c] = t

                for c in range(4):
                    fetch_chunk(c)

                # ---- h natural via PE transposes (psV ring as scratch)
                # fp8 so the SAGE matmul can run in DoubleRow (2x) perf mode
                h_nat = natp.tile([P, NT8, D], FP8, tag="hnat", name="hnat")
                for nt in range(NT8):
                    for dt in range(DT2):
                        pst = psS.tile([P, NPG], F32, space="PSUM", tag="s", name="s")
                        nc.tensor.transpose(
                            out=pst[:, 0:P],
                            in_=hT_f[dt][:, nt * P:(nt + 1) * P],
                            identity=ident[:],
                        )
                        nc.vector.tensor_copy(out=h_nat[:, nt, dt * P:(dt + 1) * P],
                                              in_=pst[:, 0:P])

                # ---- Q/K projections (d-major) ----
                QT = [qkp.tile([P, NPG], BF16, tag=f"QT{dt}", name=f"QT{dt}")
                      for dt in range(DT2)]
                KT = [qkp.tile([P, NPG], BF16, tag=f"KT{dt}", name=f"KT{dt}")
                      for dt in range(DT2)]
                for dst, w_t, b_idx in ((QT, wqT[l], 1), (KT, wkT[l], 2)):
                    for dt in range(DT2):
                        def drain_qk(ps, s0, w, dst=dst, dt=dt, b_idx=b_idx):
                            nc.vector.tensor_scalar(
                                out=dst[dt][:, s0:s0 + w], in0=ps[:, 0:w],
                                scalar1=bias_ap(l, b_idx, dt), scalar2=None,
                                op0=ALU.add)
                        mm_slices(
                            [w_t[:, kt, dt * P:(dt + 1) * P] for kt in range(DT2)],
                            [hT_b[kt][:] for kt in range(DT2)], NPG, drain_qk)
                # stage head-3 rows (base partition 96 not addressable by PE lhsT)
                q_stg = [qkp.tile([DH, NPG], BF16, tag=f"qstg{dt}", name=f"qstg{dt}")
                         for dt in range(DT2)]
                k_stg = [qkp.tile([DH, NPG], BF16, tag=f"kstg{dt}", name=f"kstg{dt}")
                         for dt in range(DT2)]
                for dt in range(DT2):
                    nc.vector.tensor_copy(out=q_stg[dt][:], in_=QT[dt][96:128, :])
                    nc.vector.tensor_copy(out=k_stg[dt][:], in_=KT[dt][96:128, :])

                # ---- V natural per node tile with ones column (emitted inside
                # head-0's score slots, using the then-idle psV bank) ----
                Vn = [qkp.tile([P, H, DH + 1], BF16, tag=f"Vn{nt}", name=f"Vn{nt}")
                      for nt in range(NT8)]

                def emit_v(nt):
                    psv = psV.tile([P, 512], F32, space="PSUM", tag="v", name="v")
                    nc.tensor.matmul(out=psv[:, 0:D], lhsT=ones_row[:],
                                     rhs=vbr[l][:], start=True, stop=False)
                    for kt in range(DT2):
                        nc.tensor.matmul(
                            out=psv[:, 0:D],
                            lhsT=hT_b[kt][:, nt * P:(nt + 1) * P],
                            rhs=wvT[l][:, kt, :],
                            start=False, stop=(kt == DT2 - 1),
                        )
                    nc.vector.tensor_copy(out=Vn[nt][:, :, 0:DH], in_=psv[:, 0:D])
                    nc.vector.memset(Vn[nt][:, :, DH:DH + 1], 1.0)

                # ---- main interleave: attention scores/exp/PV + SAGE chunks ----
                scale = 1.0 / np.sqrt(DH)
                O_nat = onp.tile([P, NT8, D], F32, tag="onat", name="onat")
                agg_sb = [natp.tile([P, NPG], BF16, tag=f"aggsb{dt}",
                                    name=f"aggsb{dt}") for dt in range(DT2)]
                cc_rs_in = dram.tile([NCORES, DT2, P, NPG], BF16, tag="rsin",
                                     name="rsin")
                cc_rs_out = dram.tile([DT2, P, NPG], BF16, tag="rsout", name="rsout")

                # SAGE chunk emission state
                sage_state = {"next": 0, "mm": 0, "ps": None}

                def emit_sage_mms(n):
                    # emit up to n SAGE DoubleRow matmuls (kt pairs x dt)
                    for _ in range(n):
                        c = sage_state["next"]
                        if c >= CH:
                            return
                        if sage_state["mm"] == 0:
                            if at_tiles[c] is None:
                                fetch_chunk(c)
                            sage_state["ps"] = [
                                psA.tile([P, 512], F32, space="PSUM",
                                         tag="a", name="a")
                                for _ in range(DT2)]
                        i = sage_state["mm"]
                        j, dt = i // DT2, i % DT2
                        nc.tensor.matmul(
                            out=sage_state["ps"][dt][:],
                            lhsT=h_nat[:, 2 * j:2 * j + 2, dt * P:(dt + 1) * P],
                            rhs=at_tiles[c][:, 2 * j:2 * j + 2, :],
                            start=(j == 0), stop=(j == NT8 // 2 - 1),
                            perf_mode=mybir.MatmulPerfMode.DoubleRow,
                        )
                        sage_state["mm"] += 1
                        if sage_state["mm"] == NT8 // 2 * DT2:
                            # chunk complete: drain both dt planes + stage out
                            st = stg.tile([P, DT2, CHW], BF16, tag="stg", name="stg")
                            iv = invd_bc[:, c * CHW:(c + 1) * CHW]
                            nc.vector.tensor_tensor(out=st[:, 0, :], op=ALU.mult,
                                                    in0=sage_state["ps"][0][:], in1=iv)
                            nc.vector.tensor_tensor(out=st[:, 1, :], op=ALU.mult,
                                                    in0=sage_state["ps"][1][:], in1=iv)
                            cc, hh = c // 2, c % 2
                            nc.sync.dma_start(
                                out=cc_rs_in[cc, :, :, hh * CHW:(hh + 1) * CHW]
                                .rearrange("t p f -> p t f"),
                                in_=st[:])
                            if c + 4 < CH:
                                fetch_chunk(c + 4)
                            sage_state["next"] = c + 1
                            sage_state["mm"] = 0

                def emit_pv_group(hp, exp_p, pv, qt):
                    for kt in range(NT8):
                        nc.tensor.matmul(
                            out=pv[:, qt * 64:qt * 64 + DH + 1],
                            lhsT=exp_p[kt][:, qt * P:(qt + 1) * P],
                            rhs=Vn[kt][:, hp, :],
                            start=(kt == 0), stop=(kt == NT8 - 1),
                        )

                def emit_pv_norm(hp, pv):
                    # batched reciprocal of the 8 denominators (col 32+64j)
                    pv_ap = pv[:]
                    den = bass.AP(tensor=pv_ap.tensor, offset=pv_ap.offset + DH,
                                  ap=[list(pv_ap.ap[0])] + [[64, NT8]])
                    rs_h = onp.tile([P, NT8], F32, tag=f"rs{hp % 2}",
                                    name=f"rs{hp % 2}")
                    nc.vector.reciprocal(out=rs_h[:], in_=den)
                    for qt in range(NT8):
                        nc.vector.tensor_scalar(
                            out=O_nat[:, qt, hp * DH:(hp + 1) * DH],
                            in0=pv[:, qt * 64:qt * 64 + DH],
                            scalar1=rs_h[:, qt:qt + 1], scalar2=None,
                            op0=ALU.mult)

                # PV of head h-1 is threaded through head h's score slots so
                # the PE never lumps 64 PV matmuls at a head boundary
                expt, expt_prev, pv_prev = {}, None, None
                for h in range(H):
                    qdt, qr = h // 4, DH * (h % 4)
                    q_src = QT[qdt] if qr < 96 else q_stg[qdt]
                    k_src = KT[qdt] if qr < 96 else k_stg[qdt]
                    qb_, qe_ = (qr, qr + DH) if qr < 96 else (0, DH)
                    for kt in range(NT8):
                        et = expp.tile([P, NPG], BF16, tag="expt", name="expt")
                        ps_sc = psS.tile([P, NPG], F32, space="PSUM",
                                         tag="s", name="s")
                        for s in range(2):
                            nc.tensor.matmul(
                                out=ps_sc[:, s * 512:(s + 1) * 512],
                                lhsT=k_src[qb_:qe_, kt * P:(kt + 1) * P],
                                rhs=q_src[qb_:qe_, s * 512:(s + 1) * 512],
                                start=True, stop=True,
                            )
                            emit_sage_mms(2)
                        nc.scalar.activation(out=et[:], in_=ps_sc[:],
                                             func=AF.Exp, scale=scale)
                        if h == 0:
                            emit_v(kt)
                        else:
                            emit_pv_group(h - 1, expt_prev, pv_prev, kt)
                            if kt == NT8 - 1:
                                emit_pv_norm(h - 1, pv_prev)
                        expt[kt] = et
                    expt_prev, expt = expt, {}
                    pv_prev = psV.tile([P, 512], F32, space="PSUM",
                                       tag="v", name="v")
                    emit_sage_mms(4)
                # drain the last head's PV
                for qt in range(NT8):
                    emit_pv_group(H - 1, expt_prev, pv_prev, qt)
                emit_pv_norm(H - 1, pv_prev)

                # ---- finish any remaining SAGE work, then ReduceScatter ----
                emit_sage_mms(CH * NT8 * DT2)
                nc.gpsimd.collective_compute(
                    "ReduceScatter", ALU.add, replica_groups=RG,
                    ins=[cc_rs_in[:].opt()], outs=[cc_rs_out[:].opt()],
                )
                nc.sync.dma_start(
                    out=agg_sb[0][:, 0:NPG], in_=cc_rs_out[0, :, :])
                nc.sync.dma_start(
                    out=agg_sb[1][:, 0:NPG], in_=cc_rs_out[1, :, :])

                # ---- O transposes to d-major + out projection -> x2 ----
                OT = [onp.tile([P, NPG], BF16, tag=f"OT{dt}", name=f"OT{dt}")
                      for dt in range(DT2)]
                for qt in range(NT8):
                    for dt in range(DT2):
                        pst = psS.tile([P, NPG], F32, space="PSUM", tag="s", name="s")
                        nc.tensor.transpose(
                            out=pst[:, 0:P],
                            in_=O_nat[:, qt, dt * P:(dt + 1) * P],
                            identity=ident[:],
                        )
                        nc.vector.tensor_copy(out=OT[dt][:, qt * P:(qt + 1) * P],
                                              in_=pst[:, 0:P])

                x2T = [xp.tile([P, NPG], F32, tag=f"x2T{dt}", name=f"x2T{dt}")
                       for dt in range(DT2)]
                for dt in range(DT2):
                    def drain_o(ps, s0, w, dt=dt):
                        nc.vector.scalar_tensor_tensor(
                            out=x2T[dt][:, s0:s0 + w], in0=ps[:, 0:w],
                            scalar=bias_ap(l, 3, dt),
                            in1=hT_f[dt][:, s0:s0 + w],
                            op0=ALU.add, op1=ALU.add)
                    mm_slices(
                        [owT[l][:, kt, dt * P:(dt + 1) * P] for kt in range(DT2)],
                        [OT[kt][:] for kt in range(DT2)], NPG, drain_o)

                # x2 stats up-front: x2 is ready before the RS result lands,
                # so these ops must precede the x1 drains in queue order.
                # Raw moments (sum x, sum x^2) via stt accumulators, dt0 on
                # gpsimd / dt1 on DVE so the two halves run in parallel.
                stats = small.tile([P, 8], F32, tag="stats", name="stats")
                outf = [xp.tile([P, NPG], F32, tag=f"outf{dt}", name=f"outf{dt}")
                        for dt in range(DT2)]
                out_b = [xp.tile([P, NPG], BF16, tag=f"outb{dt}", name=f"outb{dt}")
                         for dt in range(DT2)]
                tmpf = xp.tile([P, NPG], F32, tag="tmpf", name="tmpf")

                def emit_stats(xt, dt, c):
                    # raw moments; dt0 on ACT (Identity/Square are in every
                    # activation table - no table thrash), dt1 on DVE
                    scr = tmpf if dt else outf[0]
                    if dt == 0:
                        nc.scalar.activation(out=scr[:], in_=xt[dt][:],
                                             func=AF.Identity,
                                             accum_out=stats[:, c:c + 1])
                        nc.scalar.activation(out=scr[:], in_=xt[dt][:],
                                             func=AF.Square,
                                             accum_out=stats[:, c + 1:c + 2])
                    else:
                        nc.vector.scalar_tensor_tensor(
                            out=scr[:], in0=xt[dt][:], scalar=0.0, in1=xt[dt][:],
                            op0=ALU.mult, op1=ALU.add,
                            accum_out=stats[:, c:c + 1])
                        nc.vector.scalar_tensor_tensor(
                            out=scr[:], in0=xt[dt][:], scalar=1.0, in1=xt[dt][:],
                            op0=ALU.mult, op1=ALU.mult,
                            accum_out=stats[:, c + 1:c + 2])

                for dt in range(DT2):
                    emit_stats(x2T, dt, 4 + dt * 2)

                # ---- SAGE local transform -> x1 (needs RS result) ----
                x1T = [xp.tile([P, NPG], F32, tag=f"x1T{dt}", name=f"x1T{dt}")
                       for dt in range(DT2)]
                for dt in range(DT2):
                    def drain_x1(ps, s0, w, dt=dt):
                        nc.vector.scalar_tensor_tensor(
                            out=x1T[dt][:, s0:s0 + w], in0=ps[:, 0:w],
                            scalar=bias_ap(l, 0, dt),
                            in1=hT_f[dt][:, s0:s0 + w],
                            op0=ALU.add, op1=ALU.add)
                    # wr@h terms first: they only need h, so the PE can start
                    # while the ReduceScatter readback is still landing
                    lhs = ([wrT[l][:, kt, dt * P:(dt + 1) * P] for kt in range(DT2)]
                           + [wlT[l][:, kt, dt * P:(dt + 1) * P] for kt in range(DT2)])
                    rhs = [hT_b[kt][:] for kt in range(DT2)] \
                        + [agg_sb[kt][:] for kt in range(DT2)]
                    mm_slices(lhs, rhs, NPG, drain_x1)

                # ---- BN stats for n1 (x1), then the joint AllGather ----
                for dt in range(DT2):
                    emit_stats(x1T, dt, dt * 2)
                cc_in = dram.tile([P, 8], F32, tag="r1in", name="r1in")
                cc_out = dram.tile([NCORES, P, 8], F32, tag="r1out", name="r1out",
                                   addr_space="Shared")
                nc.sync.dma_start(out=cc_in[:], in_=stats[:])
                nc.gpsimd.collective_compute(
                    "AllGather", ALU.bypass, replica_groups=RG,
                    ins=[cc_in[:].opt()], outs=[cc_out[:].opt()],
                )
                gsum = small.tile([P, NCORES, 8], F32, tag="gsum", name="gsum")
                nc.sync.dma_start(out=gsum[:],
                                  in_=cc_out[:].rearrange("r p s -> p r s"))
                nc.vector.tensor_add(out=gsum[:, 0:4, :], in0=gsum[:, 0:4, :],
                                     in1=gsum[:, 4:8, :])
                nc.vector.tensor_add(out=gsum[:, 0:2, :], in0=gsum[:, 0:2, :],
                                     in1=gsum[:, 2:4, :])
                nc.vector.tensor_add(out=gsum[:, 0, :], in0=gsum[:, 0, :],
                                     in1=gsum[:, 1, :])
                gm = small.tile([P, 8], F32, tag="gm", name="gm")
                nc.vector.tensor_scalar(out=gm[:], in0=gsum[:, 0, :],
                                        scalar1=1.0 / N, scalar2=None,
                                        op0=ALU.mult)

                # batched scale/shift for n1 (cols 0,1) and n2 (cols 2,3), per dt
                def gap(t, off, n, stride):
                    a = t[:]
                    return bass.AP(tensor=a.tensor, offset=a.offset + off,
                                   ap=[list(a.ap[0])] + [[stride, n]])
                m4, e4 = gap(gm, 0, 4, 2), gap(gm, 1, 4, 2)
                var4 = small.tile([P, 4], F32, tag="var4", name="var4")
                sc4 = small.tile([P, 4], F32, tag="sc4", name="sc4")
                t4 = small.tile([P, 4], F32, tag="t4", name="t4")
                nc.vector.tensor_tensor(out=var4[:], in0=m4, in1=m4, op=ALU.mult)
                nc.vector.tensor_tensor(out=var4[:], in0=e4, in1=var4[:],
                                        op=ALU.subtract)
                nc.scalar.activation(out=var4[:], in_=var4[:], func=AF.Sqrt,
                                     bias=eps_t[:])
                nc.vector.reciprocal(out=var4[:], in_=var4[:])
                # w/b for (n1,dt0),(n1,dt1),(n2,dt0),(n2,dt1): nrm idx 0,2 / 1,3
                nv = nrmv[l][:]
                w4 = bass.AP(tensor=nv.tensor, offset=nv.offset,
                             ap=[list(nv.ap[0])] + [[4, 2], [1, 2]])
                b4 = bass.AP(tensor=nv.tensor, offset=nv.offset + 2,
                             ap=[list(nv.ap[0])] + [[4, 2], [1, 2]])
                nc.vector.tensor_tensor(out=sc4[:], in0=var4[:],
                                        in1=w4, op=ALU.mult)
                nc.vector.tensor_tensor(out=t4[:], in0=m4, in1=sc4[:], op=ALU.mult)
                nc.vector.tensor_tensor(out=t4[:], in0=b4, in1=t4[:],
                                        op=ALU.subtract)
                tc2 = small.tile([P, 2], F32, tag="tc2", name="tc2")
                nc.vector.tensor_add(out=tc2[:], in0=t4[:, 0:2], in1=t4[:, 2:4])

                # ---- out = n1(x1) + n2(x2) ----
                for dt in range(DT2):
                    for s in range(2):
                        sl = slice(s * 512, (s + 1) * 512)
                        nc.scalar.activation(out=outf[dt][:, sl], in_=x1T[dt][:, sl],
                                             func=AF.Identity,
                                             scale=sc4[:, dt:dt + 1],
                                             bias=tc2[:, dt:dt + 1])
                        nc.vector.scalar_tensor_tensor(
                            out=out_b[dt][:, sl], in0=x2T[dt][:, sl],
                            scalar=sc4[:, 2 + dt:3 + dt], in1=outf[dt][:, sl],
                            op0=ALU.mult, op1=ALU.add)

                # ---- MLP residual ----
                relu1 = [qkp.tile([P, NPG], BF16, tag=f"relu1{ft}", name=f"relu1{ft}")
                         for ft in range(FT4)]
                for ft in range(FT4):
                    def drain_r(ps, s0, w, ft=ft):
                        nc.scalar.activation(out=relu1[ft][:, s0:s0 + w],
                                             in_=ps[:, 0:w], func=AF.Relu,
                                             bias=b1v[l][:, ft:ft + 1])
                    mm_slices(
                        [w1T[l][:, kt, ft * P:(ft + 1) * P] for kt in range(DT2)],
                        [out_b[kt][:] for kt in range(DT2)], NPG, drain_r)
                out2 = [xp.tile([P, NPG], F32, tag=f"out2{dt}", name=f"out2{dt}")
                        for dt in range(DT2)]
                stats3 = small.tile([P, 4], F32, tag="stats3", name="stats3")
                for dt in range(DT2):
                    def drain_m(ps, s0, w, dt=dt):
                        nc.vector.scalar_tensor_tensor(
                            out=out2[dt][:, s0:s0 + w], in0=ps[:, 0:w],
                            scalar=bias_ap(l, 4, dt),
                            in1=out_b[dt][:, s0:s0 + w],
                            op0=ALU.add, op1=ALU.add)
                    mm_slices(
                        [w2T[l][:, kt, dt * P:(dt + 1) * P] for kt in range(FT4)],
                        [relu1[kt][:] for kt in range(FT4)], NPG, drain_m)
                for dt in range(DT2):
                    scr = tmpf if dt else outf[0]
                    c = dt * 2
                    if dt == 0:
                        nc.scalar.activation(out=scr[:], in_=out2[dt][:],
                                             func=AF.Identity,
                                             accum_out=stats3[:, c:c + 1])
                        nc.scalar.activation(out=scr[:], in_=out2[dt][:],
                                             func=AF.Square,
                                             accum_out=stats3[:, c + 1:c + 2])
                    else:
                        nc.vector.scalar_tensor_tensor(
                            out=scr[:], in0=out2[dt][:], scalar=0.0,
                            in1=out2[dt][:], op0=ALU.mult, op1=ALU.add,
                            accum_out=stats3[:, c:c + 1])
                        nc.vector.scalar_tensor_tensor(
                            out=scr[:], in0=out2[dt][:], scalar=1.0,
                            in1=out2[dt][:], op0=ALU.mult, op1=ALU.mult,
                            accum_out=stats3[:, c + 1:c + 2])
                cc3_in = dram.tile([P, 4], F32, tag="r2in", name="r2in")
                cc3_out = dram.tile([NCORES, P, 4], F32, tag="r2out", name="r2out",
                                    addr_space="Shared")
                nc.sync.dma_start(out=cc3_in[:], in_=stats3[:])
                nc.gpsimd.collective_compute(
                    "AllGather", ALU.bypass, replica_groups=RG,
                    ins=[cc3_in[:].opt()], outs=[cc3_out[:].opt()],
                )
                gsum3 = small.tile([P, NCORES, 4], F32, tag="gsum3", name="gsum3")
                nc.sync.dma_start(out=gsum3[:],
                                  in_=cc3_out[:].rearrange("r p s -> p r s"))
                nc.vector.tensor_add(out=gsum3[:, 0:4, :], in0=gsum3[:, 0:4, :],
                                     in1=gsum3[:, 4:8, :])
                nc.vector.tensor_add(out=gsum3[:, 0:2, :], in0=gsum3[:, 0:2, :],
                                     in1=gsum3[:, 2:4, :])
                nc.vector.tensor_add(out=gsum3[:, 0, :], in0=gsum3[:, 0, :],
                                     in1=gsum3[:, 1, :])
                g3 = small.tile([P, 4], F32, tag="g3", name="g3")
                nc.vector.tensor_scalar(out=g3[:], in0=gsum3[:, 0, :],
                                        scalar1=1.0 / N, scalar2=None,
                                        op0=ALU.mult)
                # batched over dt: m3 = cols 0,2 ; e3 = cols 1,3
                m2_, e2_ = gap(g3, 0, 2, 2), gap(g3, 1, 2, 2)
                v2 = small.tile([P, 2], F32, tag="v2", name="v2")
                r2 = small.tile([P, 2], F32, tag="r2", name="r2")
                al2 = small.tile([P, 2], F32, tag="al2", name="al2")
                be2 = small.tile([P, 2], F32, tag="be2", name="be2")
                nc.vector.tensor_tensor(out=v2[:], in0=m2_, in1=m2_, op=ALU.mult)
                nc.vector.tensor_tensor(out=v2[:], in0=e2_, in1=v2[:],
                                        op=ALU.subtract)
                nc.scalar.activation(out=r2[:], in_=v2[:], func=AF.Sqrt,
                                     bias=eps_t[:])
                nc.vector.reciprocal(out=r2[:], in_=r2[:])
                w3_ = bass.AP(tensor=nv.tensor, offset=nv.offset + 4 * 2,
                              ap=[list(nv.ap[0])] + [[1, 2]])   # n3_w per dt
                bw_ = bass.AP(tensor=nv.tensor, offset=nv.offset + 6 * 2,
                              ap=[list(nv.ap[0])] + [[1, 2]])   # bn_w per dt
                bb_ = bass.AP(tensor=nv.tensor, offset=nv.offset + 7 * 2,
                              ap=[list(nv.ap[0])] + [[1, 2]])   # bn_b per dt
                # al = w3*r3; rbn = rsqrt(al^2*v3+eps); al = al*rbn*bw; be = bb-m3*al
                nc.vector.tensor_tensor(out=al2[:], in0=w3_, in1=r2[:], op=ALU.mult)
                nc.vector.tensor_tensor(out=be2[:], in0=al2[:], in1=al2[:],
                                        op=ALU.mult)
                nc.vector.tensor_tensor(out=be2[:], in0=be2[:], in1=v2[:],
                                        op=ALU.mult)
                nc.scalar.activation(out=be2[:], in_=be2[:], func=AF.Sqrt,
                                     bias=eps_t[:])
                nc.vector.reciprocal(out=be2[:], in_=be2[:])
                nc.vector.tensor_tensor(out=al2[:], in0=al2[:], in1=be2[:],
                                        op=ALU.mult)
                nc.vector.tensor_tensor(out=al2[:], in0=al2[:], in1=bw_, op=ALU.mult)
                nc.vector.tensor_tensor(out=be2[:], in0=m2_, in1=al2[:], op=ALU.mult)
                nc.vector.tensor_tensor(out=be2[:], in0=bb_, in1=be2[:],
                                        op=ALU.subtract)
                hT_f_new = [featp.tile([P, NPG], F32, tag=f"hTf{dt}", name=f"hTf{dt}")
                            for dt in range(DT2)]
                hT_b_new = [featp.tile([P, NPG], BF16, tag=f"hTb{dt}",
                                       name=f"hTb{dt}") for dt in range(DT2)]
                for dt in range(DT2):
                    for s in range(2):
                        sl = slice(s * 512, (s + 1) * 512)
                        nc.scalar.activation(out=tmpf[:, sl], in_=out2[dt][:, sl],
                                             func=AF.Relu,
                                             scale=al2[:, dt:dt + 1],
                                             bias=be2[:, dt:dt + 1])
                        nc.vector.tensor_add(out=hT_f_new[dt][:, sl],
                                             in0=hT_f[dt][:, sl], in1=tmpf[:, sl])
                    if l < L - 1:  # bf16 h only feeds next layer's matmuls
                        nc.gpsimd.tensor_copy(out=hT_b_new[dt][:],
                                              in_=hT_f_new[dt][:])
                hT_f, hT_b = hT_f_new, hT_b_new

            # ---------------- pool + head ----------------
            pooled = small.tile([P, DT2], F32, tag="pooled", name="pooled")
            pooled_b = small.tile([P, DT2], BF16, tag="pooledb", name="pooledb")
            for dt in range(DT2):
                nc.vector.tensor_reduce(out=pooled[:, dt:dt + 1], in_=hT_f[dt][:],
                                        axis=mybir.AxisListType.X, op=ALU.add)
            nc.scalar.activation(out=pooled_b[:], in_=pooled[:], func=AF.Identity,
                                 scale=1.0 / NPG)
            ps_y = psA.tile([P, 512], F32, space="PSUM", tag="a", name="a")
            for dt in range(DT2):
                nc.tensor.matmul(out=ps_y[0:OUT_D, 0:1],
                                 lhsT=w_outT[:, dt, :],
                                 rhs=pooled_b[:, dt:dt + 1],
                                 start=(dt == 0), stop=(dt == DT2 - 1))
            y_sb = small.tile([OUT_D, 1], F32, tag="ysb", name="ysb")
            nc.scalar.activation(out=y_sb[:], in_=ps_y[0:OUT_D, 0:1],
                                 func=AF.Identity, bias=boutv[:])
            nc.sync.dma_start(out=y_out[:, :], in_=y_sb[:])

    return nc


# ---------------------------------------------------------------------------
# Host-side: shard inputs, run, gather
# ---------------------------------------------------------------------------
def prep_inputs(x, edge_index, batch, w_in, b_in, sage_wl, sage_bl, sage_wr,
                attn_iw, attn_ib, attn_ow, attn_ob, n1_w, n1_b, n2_w, n2_b,
                n3_w, n3_b, mlp_w1, mlp_b1, mlp_w2, mlp_b2, bn_w, bn_b,
                w_out, b_out):
    bf = ml_dtypes.bfloat16
    f8 = ml_dtypes.float8_e4m3
    x = np.asarray(x, np.float32)
    ei = np.asarray(edge_index)
    src, dst = np.asarray(ei[0], np.int64), np.asarray(ei[1], np.int64)
    deg = np.bincount(dst, minlength=N).astype(np.float32)
    inv_deg = 1.0 / np.clip(deg, 1.0, None)

    def t32(a):
        return np.ascontiguousarray(np.asarray(a, np.float32))

    def packT(w_l):  # [out, in] -> [K=in/P, P, out] (transposed, packed)
        wt = t32(w_l).T  # [in, out]
        return wt.reshape(wt.shape[0] // P, P, wt.shape[1])

    shared = {
        "w_inT": t32(w_in).T.astype(bf),                       # [128, 256]
        "w_outT": packT(w_out).astype(bf),                     # [2, 128, 64]
        "wlT": np.stack([packT(sage_wl[l]) for l in range(L)]).astype(bf),
        "wrT": np.stack([packT(sage_wr[l]) for l in range(L)]).astype(bf),
        "wqT": np.stack([packT(attn_iw[l][0:D]) for l in range(L)]).astype(bf),
        "wkT": np.stack([packT(attn_iw[l][D:2 * D]) for l in range(L)]).astype(bf),
        "wvT": np.stack([packT(attn_iw[l][2 * D:3 * D]) for l in range(L)]).astype(bf),
        "owT": np.stack([packT(attn_ow[l]) for l in range(L)]).astype(bf),
        "w1T": np.stack([packT(mlp_w1[l]) for l in range(L)]).astype(bf),
        "w2T": np.stack([packT(mlp_w2[l]) for l in range(L)]).astype(bf),
        "vbr": np.stack([t32(attn_ib[l][2 * D:3 * D])[None, :]
                         for l in range(L)]).astype(bf),
        "b1v": np.stack([t32(mlp_b1[l]).reshape(FT4, P) for l in range(L)]),
        "boutv": t32(b_out)[:, None],
    }
    biasv = np.zeros((L, 8, DT2, P), np.float32)
    nrmv = np.zeros((L, 8, DT2, P), np.float32)
    for l in range(L):
        biasv[l, 0] = t32(sage_bl[l]).reshape(DT2, P)
        biasv[l, 1] = t32(attn_ib[l][0:D]).reshape(DT2, P)
        biasv[l, 2] = t32(attn_ib[l][D:2 * D]).reshape(DT2, P)
        biasv[l, 3] = t32(attn_ob[l]).reshape(DT2, P)
        biasv[l, 4] = t32(mlp_b2[l]).reshape(DT2, P)
        if l == 0:
            biasv[l, 5] = t32(b_in).reshape(DT2, P)
        nrmv[l, 0] = t32(n1_w[l]).reshape(DT2, P)
        nrmv[l, 1] = t32(n1_b[l]).reshape(DT2, P)
        nrmv[l, 2] = t32(n2_w[l]).reshape(DT2, P)
        nrmv[l, 3] = t32(n2_b[l]).reshape(DT2, P)
        nrmv[l, 4] = t32(n3_w[l]).reshape(DT2, P)
        nrmv[l, 5] = t32(n3_b[l]).reshape(DT2, P)
        nrmv[l, 6] = t32(bn_w[l]).reshape(DT2, P)
        nrmv[l, 7] = t32(bn_b[l]).reshape(DT2, P)
    shared["biasv"] = biasv
    shared["nrmv"] = nrmv

    in_maps = []
    for c in range(NCORES):
        lo, hi = c * NPG, (c + 1) * NPG
        sel = (src >= lo) & (src < hi)
        s_c, d_c = src[sel] - lo, dst[sel]
        at = np.zeros(NPG * N, np.float32)
        np.add.at(at, s_c * N + d_c, 1.0)
        m = dict(shared)
        m["xT"] = np.ascontiguousarray(x[lo:hi].T).astype(bf)
        m["at"] = at.reshape(NPG, N).astype(f8)
        m["invd"] = inv_deg[None, :].astype(bf)
        in_maps.append(m)
    return in_maps


_NC_CACHE = {}


def get_nc():
    if "nc" not in _NC_CACHE:
        _NC_CACHE["nc"] = build_kernel()
    return _NC_CACHE["nc"]


def kernel(**inputs):
    in_maps = prep_inputs(**inputs)
    nc = get_nc()
    res = run_bass_kernel_spmd(nc, in_maps, list(range(NCORES)))
    out = np.stack([res.results[c]["y"][:, 0] for c in range(NCORES)])
    return out.astype(np.float32)


if __name__ == "__main__":
    rng = np.random.default_rng(0)
    ins = dict(
        x=rng.standard_normal((N, IN_C), dtype=np.float32),
        edge_index=rng.integers(0, N, (2, E)),
        batch=np.arange(N, dtype=np.int32) // NPG,
        w_in=rng.standard_normal((D, IN_C), dtype=np.float32) * 0.05,
        b_in=rng.standard_normal(D, dtype=np.float32) * 0.05,
        sage_wl=rng.standard_normal((L, D, D), dtype=np.float32) * 0.05,
        sage_bl=rng.standard_normal((L, D), dtype=np.float32) * 0.05,
        sage_wr=rng.standard_normal((L, D, D), dtype=np.float32) * 0.05,
        attn_iw=rng.standard_normal((L, 3 * D, D), dtype=np.float32) * 0.05,
        attn_ib=rng.standard_normal((L, 3 * D), dtype=np.float32) * 0.05,
        attn_ow=rng.standard_normal((L, D, D), dtype=np.float32) * 0.05,
        attn_ob=rng.standard_normal((L, D), dtype=np.float32) * 0.05,
        n1_w=np.ones((L, D), np.float32), n1_b=np.zeros((L, D), np.float32),
        n2_w=np.ones((L, D), np.float32), n2_b=np.zeros((L, D), np.float32),
        n3_w=np.ones((L, D), np.float32), n3_b=np.zeros((L, D), np.float32),
        mlp_w1=rng.standard_normal((L, DFF, D), dtype=np.float32) * 0.05,
        mlp_b1=rng.standard_normal((L, DFF), dtype=np.float32) * 0.05,
        mlp_w2=rng.standard_normal((L, D, DFF), dtype=np.float32) * 0.05,
        mlp_b2=rng.standard_normal((L, D), dtype=np.float32) * 0.05,
        bn_w=np.ones((L, D), np.float32), bn_b=np.zeros((L, D), np.float32),
        w_out=rng.standard_normal((OUT_D, D), dtype=np.float32) * 0.05,
        b_out=rng.standard_normal(OUT_D, dtype=np.float32) * 0.05,
    )
    y = kernel(**ins)
    print("y shape:", y.shape, "finite:", np.isfinite(y).all())


# revision 51
# speedup vs baseline: 1.0408x; 1.0408x over previous
"""GPS (GraphGPS) forward pass on 8 Trainium2 NeuronCores.

Model (from the reference): 2 layers of
  SAGEConv(mean aggr) + residual + BN  ||  per-graph dense MHA + residual + BN
  -> sum branches -> MLP residual -> BN -> outer BN + relu + residual
then per-graph mean pool + linear head.

Sharding: one graph (1024 nodes) per core. The SAGE neighbor aggregation is
computed ReduceScatter-style: each core multiplies its LOCAL node features
h_c [1024, 256] against its src-slice of the dense edge-count matrix
A_c [1024 src x 8192 dst] (fp8 counts, exact small ints), producing partial
aggregates for ALL destinations; a ReduceScatter(add) then hands every core
the summed aggregate rows for its own 1024 destinations, which are scaled by
1/deg locally. This needs no AllGather of features at all. BatchNorm batch
stats are exchanged with small AllGathers (cheaper than AllReduce here) and
summed locally.

Device layout: features kept transposed (hT = [256 dims x 1024 nodes], dims
on partitions) so BN stats/apply are per-partition ops; h natural
([node, dim], from 16 PE transposes per layer) feeds the SAGE matmul as lhsT.
Attention: scores^T [keys, q] per (head, key-tile); exp on ACT; PV contracts
over keys with the 33-wide (V ++ ones) natural V so output lands natural
[q, d] with the softmax denominator on the same partition as its query row
(per-partition normalize), then 16 PE transposes take O back to d-major for
the out-projection.
"""
import numpy as np
import ml_dtypes

import concourse.bass as bass
import concourse.mybir as mybir
import concourse.tile as tile
from concourse.bass_utils import run_bass_kernel_spmd
from concourse.vector_clock import ScopedClock
from concourse.masks import make_identity

# ---------------------------------------------------------------------------
# Walrus workaround: this toolchain rejects >1 sync-wait command per
# instruction. Hoist excess waits onto same-engine NoOps / extra drains.
# ---------------------------------------------------------------------------
_MAX_WAITS = 1


def _split_waits_in_ordered(nc, ordered):
    for bb_name, insts in ordered.items():
        new_list = []
        for inst in insts:
            si = getattr(inst, "sync_info", None)
            if si is not None and si.on_wait and len(si.on_wait) > _MAX_WAITS:
                waits = list(si.on_wait)
                keep = waits[-_MAX_WAITS:]
                for w in waits[:-_MAX_WAITS]:
                    nop = mybir.InstNoOp(
                        name=nc.get_next_instruction_name(),
                        engine=inst.engine,
                        ins=[],
                        outs=[],
                        sync_info=mybir.SyncInfo(on_wait=[w], on_update=[]),
                    )
                    nop.debug = inst.debug
                    new_list.append(nop)
                si.on_wait[:] = keep
            new_list.append(inst)
        insts[:] = new_list


_orig_lower = tile.TileContext._lower_ordered_insts


def _patched_lower_ordered_insts(self, ordered):
    _split_waits_in_ordered(self.nc, ordered)
    return _orig_lower(self, ordered)


def _patched_drain_and_barrier(self, tick_clock, wait_clock):
    drain_inst = self.nc.sync.drain()
    wait_clock.add_sem_waits(drain_inst.ins, ScopedClock({None: tick_clock.global_clock}))
    si = drain_inst.ins.sync_info
    waits = list(si.on_wait) if si is not None else []
    if len(waits) > _MAX_WAITS:
        si.on_wait[:] = waits[:_MAX_WAITS]
        for w in waits[_MAX_WAITS:]:
            d2 = self.nc.sync.drain()
            d2.ins.sync_info = mybir.SyncInfo(on_wait=[w], on_update=[])
    self.nc.all_engine_barrier()
    assert self.sems is not None
    popped = self.nc._tile_sem_poison_stack.pop()
    assert popped is self._sem_poison
    self.nc.clear_and_free_semaphores(list(self.sems.allocated().values()))
    self.nc.all_engine_barrier()


tile.TileContext._lower_ordered_insts = _patched_lower_ordered_insts
tile.TileContext._drain_and_barrier = _patched_drain_and_barrier

# ---------------------------------------------------------------------------
# Problem constants (hardcoded per the task contract)
# ---------------------------------------------------------------------------
N, B, NPG = 8192, 8, 1024
D, H, DH, L = 256, 8, 32, 2
IN_C, OUT_D, E, DFF = 128, 64, 262144, 512
EPS = 1e-5
NCORES = 8
P = 128          # SBUF partitions
DT2 = D // P     # 2 dim tiles of 128
FT4 = DFF // P   # 4 ff tiles
NT8 = NPG // P   # 8 local node tiles
CH = 16          # dst chunks for the SAGE partial matmul
CHW = N // CH    # 512 dst per chunk
F32 = mybir.dt.float32
BF16 = mybir.dt.bfloat16
FP8 = mybir.dt.float8e4
AF = mybir.ActivationFunctionType
ALU = mybir.AluOpType
RG = [list(range(NCORES))]


def build_kernel():
    nc = bass.Bass()

    # ---- I/O declarations ----
    xT_in = nc.dram_tensor("xT", [P, NPG], BF16, kind="ExternalInput")
    at_in = nc.dram_tensor("at", [NPG, N], FP8, kind="ExternalInput")
    invd_in = nc.dram_tensor("invd", [1, N], BF16, kind="ExternalInput")
    # per-layer weights, host-transposed; leading dims packed for [128, ...] SBUF tiles
    wlT_in = nc.dram_tensor("wlT", [L, DT2, P, D], BF16, kind="ExternalInput")
    wrT_in = nc.dram_tensor("wrT", [L, DT2, P, D], BF16, kind="ExternalInput")
    wqT_in = nc.dram_tensor("wqT", [L, DT2, P, D], BF16, kind="ExternalInput")
    wkT_in = nc.dram_tensor("wkT", [L, DT2, P, D], BF16, kind="ExternalInput")
    wvT_in = nc.dram_tensor("wvT", [L, DT2, P, D], BF16, kind="ExternalInput")
    owT_in = nc.dram_tensor("owT", [L, DT2, P, D], BF16, kind="ExternalInput")
    w1T_in = nc.dram_tensor("w1T", [L, DT2, P, DFF], BF16, kind="ExternalInput")
    w2T_in = nc.dram_tensor("w2T", [L, FT4, P, D], BF16, kind="ExternalInput")
    w_inT_in = nc.dram_tensor("w_inT", [IN_C, D], BF16, kind="ExternalInput")
    w_outT_in = nc.dram_tensor("w_outT", [DT2, P, OUT_D], BF16, kind="ExternalInput")
    # biases / norm params, fp32; [idx, dt, p] so device holds [p, idx, dt]
    bias_in = nc.dram_tensor("biasv", [L, 8, DT2, P], F32, kind="ExternalInput")
    #   biasv[l]: 0=sage_b 1=qb 2=kb 3=ob 4=b2 5=b_in(l0) 6,7 spare
    b1_in = nc.dram_tensor("b1v", [L, FT4, P], F32, kind="ExternalInput")
    nrm_in = nc.dram_tensor("nrmv", [L, 8, DT2, P], F32, kind="ExternalInput")
    #   nrmv[l]: 0=n1_w 1=n1_b 2=n2_w 3=n2_b 4=n3_w 5=n3_b 6=bn_w 7=bn_b
    vb_in = nc.dram_tensor("vbr", [L, 1, D], BF16, kind="ExternalInput")
    bout_in = nc.dram_tensor("boutv", [OUT_D, 1], F32, kind="ExternalInput")

    y_out = nc.dram_tensor("y", [OUT_D, 1], F32, kind="ExternalOutput")

    with tile.TileContext(nc) as tc:
        with (
            tc.tile_pool(name="wpool", bufs=1) as wpool,      # persistent weights
            tc.tile_pool(name="featp", bufs=2) as featp,      # hT (old/new rotate)
            tc.tile_pool(name="natp", bufs=1) as natp,        # h natural + agg
            tc.tile_pool(name="qkp", bufs=1) as qkp,          # Q/K/V per layer
            tc.tile_pool(name="expp", bufs=15) as expp,       # exp(score) tiles
            tc.tile_pool(name="onp", bufs=1) as onp,          # O_nat / OT
            tc.tile_pool(name="xp", bufs=1) as xp,            # x1/x2/out/out2
            tc.tile_pool(name="stg", bufs=2) as stg,          # RS staging chunks
            tc.tile_pool(name="small", bufs=4) as small,      # stats etc
            tc.tile_pool(name="atp", bufs=4) as atp,          # A chunk stream
            tc.tile_pool(name="psA", bufs=2, space="PSUM") as psA,   # 2 banks
            tc.tile_pool(name="psS", bufs=2, space="PSUM") as psS,   # 4 banks
            tc.tile_pool(name="psV", bufs=1, space="PSUM") as psV,   # 1 bank
            tc.tile_pool(name="dram", bufs=2, space="DRAM") as dram,
        ):
            assert nc.vector.BN_STATS_FMAX >= 512

            # ---------------- load weights ----------------
            def load_w(shape, src_ap, name, dtype=BF16, pool=wpool):
                t = pool.tile(shape, dtype, tag=name, name=name)
                nc.sync.dma_start(out=t[:], in_=src_ap)
                return t

            # order matters: in_proj inputs + small params first so the first
            # matmuls aren't queued behind the big weight streams
            xT = load_w([P, NPG], xT_in[:, :], "xTw")
            w_inT = load_w([IN_C, D], w_inT_in[:, :], "w_inTw")
            biasv = [load_w([P, 8, DT2], bias_in[l].rearrange("i t p -> p i t"),
                            f"biasw{l}", F32) for l in range(L)]
            nrmv = [load_w([P, 8, DT2], nrm_in[l].rearrange("i t p -> p i t"),
                           f"nrmw{l}", F32) for l in range(L)]
            b1v = [load_w([P, FT4], b1_in[l].rearrange("t p -> p t"),
                          f"b1w{l}", F32) for l in range(L)]
            vbr = [load_w([1, D], vb_in[l], f"vbrw{l}") for l in range(L)]
            boutv = load_w([OUT_D, 1], bout_in[:, :], "boutw", F32)

            def load_packed(src, free, nm):
                # src [L, K, P, free] -> per-layer tiles [P, K, free]
                return [load_w([P, src.shape[1], free],
                               src[l].rearrange("k p f -> p k f"), f"{nm}{l}")
                        for l in range(L)]

            wqT = load_packed(wqT_in, D, "wqTw")
            wkT = load_packed(wkT_in, D, "wkTw")
            wvT = load_packed(wvT_in, D, "wvTw")
            owT = load_packed(owT_in, D, "owTw")
            wlT = load_packed(wlT_in, D, "wlTw")
            wrT = load_packed(wrT_in, D, "wrTw")
            w1T = load_packed(w1T_in, DFF, "w1Tw")
            w2T = load_packed(w2T_in, D, "w2Tw")
            w_outT = load_w([P, DT2, OUT_D], w_outT_in[:].rearrange("t p o -> p t o"),
                            "w_outTw")

            # global inv_deg, broadcast to all partitions: folded into the
            # SAGE partial drains (pre-ReduceScatter), so the RS result is
            # the finished mean aggregation
            invd_bc = wpool.tile([P, N], BF16, tag="invdbc", name="invdbc")
            iv_ap = invd_in[:, :]
            nc.sync.dma_start(
                out=invd_bc[:],
                in_=bass.AP(tensor=iv_ap.tensor, offset=iv_ap.offset,
                            ap=[[0, P]] + list(iv_ap.ap[1:])),
            )

            ones_row = wpool.tile([1, P], BF16)
            nc.vector.memset(ones_row[:], 1.0)
            eps_t = wpool.tile([P, 1], F32)
            nc.vector.memset(eps_t[:], EPS)
            ident = wpool.tile([P, P], F32)
            make_identity(nc, ident[:])

            def bias_ap(l, idx, dt):
                return biasv[l][:, idx, dt:dt + 1]

            def nrm_ap(l, idx, dt):
                return nrmv[l][:, idx, dt:dt + 1]

            # generic matmul into psA 512-slices with per-slice drain callback
            def mm_slices(lhsT_aps, rhs_aps, nfree, drain, slice_w=512):
                for s0 in range(0, nfree, slice_w):
                    w = min(slice_w, nfree - s0)
                    ps = psA.tile([P, 512], F32, space="PSUM", tag="a", name="a")
                    nk = len(lhsT_aps)
                    for k in range(nk):
                        nc.tensor.matmul(
                            out=ps[:, 0:w], lhsT=lhsT_aps[k],
                            rhs=rhs_aps[k][:, s0:s0 + w],
                            start=(k == 0), stop=(k == nk - 1),
                        )
                    drain(ps, s0, w)

            # ---------------- in_proj ----------------
            hT_f = [featp.tile([P, NPG], F32, tag=f"hTf{dt}", name=f"hTf{dt}")
                    for dt in range(DT2)]
            hT_b = [featp.tile([P, NPG], BF16, tag=f"hTb{dt}", name=f"hTb{dt}")
                    for dt in range(DT2)]
            for dt in range(DT2):
                def drain_in(ps, s0, w, dt=dt):
                    nc.scalar.activation(out=hT_f[dt][:, s0:s0 + w], in_=ps[:, 0:w],
                                         func=AF.Identity, bias=bias_ap(0, 5, dt))
                mm_slices([w_inT[:, dt * P:(dt + 1) * P]], [xT[:]], NPG, drain_in)
                nc.gpsimd.tensor_copy(out=hT_b[dt][:], in_=hT_f[dt][:])

            # ---------------- layers ----------------
            for l in range(L):
                # ---- A chunk prefetch (first 4; rest issued inside interleave)
                at_tiles = [None] * CH

                def fetch_chunk(c):
                    t = atp.tile([P, NT8, CHW], FP8, tag="att", name="att")
                    nc.sync.dma_start(
                        out=t[:],
                        in_=at_in[:, c * CHW:(c + 1) * CHW]
                        .rearrange("(kt p) f -> p kt f", p=P))
                    at_tiles[c] = t

                for c in range(4):
                    fetch_chunk(c)

                # ---- h natural via PE transposes (psV ring as scratch)
                # fp8 so the SAGE matmul can run in DoubleRow (2x) perf mode
                h_nat = natp.tile([P, NT8, D], FP8, tag="hnat", name="hnat")
                for nt in range(NT8):
                    for dt in range(DT2):
                        pst = psS.tile([P, NPG], F32, space="PSUM", tag="s", name="s")
                        nc.tensor.transpose(
                            out=pst[:, 0:P],
                            in_=hT_f[dt][:, nt * P:(nt + 1) * P],
                            identity=ident[:],
                        )
                        nc.vector.tensor_copy(out=h_nat[:, nt, dt * P:(dt + 1) * P],
                                              in_=pst[:, 0:P])

                # ---- Q/K projections (d-major) ----
                QT = [qkp.tile([P, NPG], BF16, tag=f"QT{dt}", name=f"QT{dt}")
                      for dt in range(DT2)]
                KT = [qkp.tile([P, NPG], BF16, tag=f"KT{dt}", name=f"KT{dt}")
                      for dt in range(DT2)]
                for dst, w_t, b_idx in ((QT, wqT[l], 1), (KT, wkT[l], 2)):
                    for dt in range(DT2):
                        def drain_qk(ps, s0, w, dst=dst, dt=dt, b_idx=b_idx):
                            nc.vector.tensor_scalar(
                                out=dst[dt][:, s0:s0 + w], in0=ps[:, 0:w],
                                scalar1=bias_ap(l, b_idx, dt), scalar2=None,
                                op0=ALU.add)
                        mm_slices(
                            [w_t[:, kt, dt * P:(dt + 1) * P] for kt in range(DT2)],
                            [hT_b[kt][:] for kt in range(DT2)], NPG, drain_qk)
                # stage head-3 rows (base partition 96 not addressable by PE lhsT)
                q_stg = [qkp.tile([DH, NPG], BF16, tag=f"qstg{dt}", name=f"qstg{dt}")
                         for dt in range(DT2)]
                k_stg = [qkp.tile([DH, NPG], BF16, tag=f"kstg{dt}", name=f"kstg{dt}")
                         for dt in range(DT2)]
                for dt in range(DT2):
                    nc.vector.tensor_copy(out=q_stg[dt][:], in_=QT[dt][96:128, :])
                    nc.vector.tensor_copy(out=k_stg[dt][:], in_=KT[dt][96:128, :])

                # ---- V natural per node tile with ones column (emitted inside
                # head-0's score slots, using the then-idle psV bank) ----
                Vn = [qkp.tile([P, H, DH + 1], BF16, tag=f"Vn{nt}", name=f"Vn{nt}")
                      for nt in range(NT8)]

                def emit_v(nt):
                    psv = psV.tile([P, 512], F32, space="PSUM", tag="v", name="v")
                    nc.tensor.matmul(out=psv[:, 0:D], lhsT=ones_row[:],
                                     rhs=vbr[l][:], start=True, stop=False)
                    for kt in range(DT2):
                        nc.tensor.matmul(
                            out=psv[:, 0:D],
                            lhsT=hT_b[kt][:, nt * P:(nt + 1) * P],
                            rhs=wvT[l][:, kt, :],
                            start=False, stop=(kt == DT2 - 1),
                        )
                    nc.vector.tensor_copy(out=Vn[nt][:, :, 0:DH], in_=psv[:, 0:D])
                    nc.vector.memset(Vn[nt][:, :, DH:DH + 1], 1.0)

                # ---- main interleave: attention scores/exp/PV + SAGE chunks ----
                scale = 1.0 / np.sqrt(DH)
                O_nat = onp.tile([P, NT8, D], F32, tag="onat", name="onat")
                agg_sb = [natp.tile([P, NPG], BF16, tag=f"aggsb{dt}",
                                    name=f"aggsb{dt}") for dt in range(DT2)]
                cc_rs_in = dram.tile([NCORES, DT2, P, NPG], BF16, tag="rsin",
                                     name="rsin")
                cc_rs_out = dram.tile([DT2, P, NPG], BF16, tag="rsout", name="rsout")

                # SAGE chunk emission state
                sage_state = {"next": 0, "mm": 0, "ps": None}

                def emit_sage_mms(n):
                    # emit up to n SAGE DoubleRow matmuls (kt pairs x dt)
                    for _ in range(n):
                        c = sage_state["next"]
                        if c >= CH:
                            return
                        if sage_state["mm"] == 0:
                            if at_tiles[c] is None:
                                fetch_chunk(c)
                            sage_state["ps"] = [
                                psA.tile([P, 512], F32, space="PSUM",
                                         tag="a", name="a")
                                for _ in range(DT2)]
                        i = sage_state["mm"]
                        j, dt = i // DT2, i % DT2
                        nc.tensor.matmul(
                            out=sage_state["ps"][dt][:],
                            lhsT=h_nat[:, 2 * j:2 * j + 2, dt * P:(dt + 1) * P],
                            rhs=at_tiles[c][:, 2 * j:2 * j + 2, :],
                            start=(j == 0), stop=(j == NT8 // 2 - 1),
                            perf_mode=mybir.MatmulPerfMode.DoubleRow,
                        )
                        sage_state["mm"] += 1
                        if sage_state["mm"] == NT8 // 2 * DT2:
                            # chunk complete: drain both dt planes + stage out
                            st = stg.tile([P, DT2, CHW], BF16, tag="stg", name="stg")
                            iv = invd_bc[:, c * CHW:(c + 1) * CHW]
                            nc.vector.tensor_tensor(out=st[:, 0, :], op=ALU.mult,
                                                    in0=sage_state["ps"][0][:], in1=iv)
                            nc.vector.tensor_tensor(out=st[:, 1, :], op=ALU.mult,
                                                    in0=sage_state["ps"][1][:], in1=iv)
                            cc, hh = c // 2, c % 2
                            nc.sync.dma_start(
                                out=cc_rs_in[cc, :, :, hh * CHW:(hh + 1) * CHW]
                                .rearrange("t p f -> p t f"),
                                in_=st[:])
                            if c + 4 < CH:
                                fetch_chunk(c + 4)
                            sage_state["next"] = c + 1
                            sage_state["mm"] = 0

                expt = {}
                for h in range(H):
                    qdt, qr = h // 4, DH * (h % 4)
                    q_src = QT[qdt] if qr < 96 else q_stg[qdt]
                    k_src = KT[qdt] if qr < 96 else k_stg[qdt]
                    qb_, qe_ = (qr, qr + DH) if qr < 96 else (0, DH)
                    for kt in range(NT8):
                        et = expp.tile([P, NPG], BF16, tag="expt", name="expt")
                        ps_sc = psS.tile([P, NPG], F32, space="PSUM",
                                         tag="s", name="s")
                        for s in range(2):
                            nc.tensor.matmul(
                                out=ps_sc[:, s * 512:(s + 1) * 512],
                                lhsT=k_src[qb_:qe_, kt * P:(kt + 1) * P],
                                rhs=q_src[qb_:qe_, s * 512:(s + 1) * 512],
                                start=True, stop=True,
                            )
                            emit_sage_mms(2)
                        nc.scalar.activation(out=et[:], in_=ps_sc[:],
                                             func=AF.Exp, scale=scale)
                        if h == 0:
                            emit_v(kt)
                        expt[kt] = et
                    # PV for head h: per q-tile, contract over key tiles
                    pv = psV.tile([P, 512], F32, space="PSUM", tag="v", name="v")
                    for qt in range(NT8):
                        for kt in range(NT8):
                            nc.tensor.matmul(
                                out=pv[:, qt * 64:qt * 64 + DH + 1],
                                lhsT=expt[kt][:, qt * P:(qt + 1) * P],
                                rhs=Vn[kt][:, h, :],
                                start=(kt == 0), stop=(kt == NT8 - 1),
                            )
                    # batched reciprocal of the 8 denominators (psum col 32+64j)
                    pv_ap = pv[:]
                    den = bass.AP(tensor=pv_ap.tensor, offset=pv_ap.offset + DH,
                                  ap=[list(pv_ap.ap[0])] + [[64, NT8]])
                    rs_h = onp.tile([P, NT8], F32, tag=f"rs{h % 2}", name=f"rs{h % 2}")
                    nc.vector.reciprocal(out=rs_h[:], in_=den)
                    for qt in range(NT8):
                        nc.vector.tensor_scalar(
                            out=O_nat[:, qt, h * DH:(h + 1) * DH],
                            in0=pv[:, qt * 64:qt * 64 + DH],
                            scalar1=rs_h[:, qt:qt + 1], scalar2=None,
                            op0=ALU.mult)
                    emit_sage_mms(4)

                # ---- finish any remaining SAGE work, then ReduceScatter ----
                emit_sage_mms(CH * NT8 * DT2)
                nc.gpsimd.collective_compute(
                    "ReduceScatter", ALU.add, replica_groups=RG,
                    ins=[cc_rs_in[:].opt()], outs=[cc_rs_out[:].opt()],
                )
                nc.sync.dma_start(
                    out=agg_sb[0][:, 0:NPG], in_=cc_rs_out[0, :, :])
                nc.sync.dma_start(
                    out=agg_sb[1][:, 0:NPG], in_=cc_rs_out[1, :, :])

                # ---- O transposes to d-major + out projection -> x2 ----
                OT = [onp.tile([P, NPG], BF16, tag=f"OT{dt}", name=f"OT{dt}")
                      for dt in range(DT2)]
                for qt in range(NT8):
                    for dt in range(DT2):
                        pst = psS.tile([P, NPG], F32, space="PSUM", tag="s", name="s")
                        nc.tensor.transpose(
                            out=pst[:, 0:P],
                            in_=O_nat[:, qt, dt * P:(dt + 1) * P],
                            identity=ident[:],
                        )
                        nc.vector.tensor_copy(out=OT[dt][:, qt * P:(qt + 1) * P],
                                              in_=pst[:, 0:P])

                x2T = [xp.tile([P, NPG], F32, tag=f"x2T{dt}", name=f"x2T{dt}")
                       for dt in range(DT2)]
                for dt in range(DT2):
                    def drain_o(ps, s0, w, dt=dt):
                        nc.vector.scalar_tensor_tensor(
                            out=x2T[dt][:, s0:s0 + w], in0=ps[:, 0:w],
                            scalar=bias_ap(l, 3, dt),
                            in1=hT_f[dt][:, s0:s0 + w],
                            op0=ALU.add, op1=ALU.add)
                    mm_slices(
                        [owT[l][:, kt, dt * P:(dt + 1) * P] for kt in range(DT2)],
                        [OT[kt][:] for kt in range(DT2)], NPG, drain_o)

                # x2 stats up-front: x2 is ready before the RS result lands,
                # so these ops must precede the x1 drains in queue order.
                # Raw moments (sum x, sum x^2) via stt accumulators, dt0 on
                # gpsimd / dt1 on DVE so the two halves run in parallel.
                stats = small.tile([P, 8], F32, tag="stats", name="stats")
                outf = [xp.tile([P, NPG], F32, tag=f"outf{dt}", name=f"outf{dt}")
                        for dt in range(DT2)]
                out_b = [xp.tile([P, NPG], BF16, tag=f"outb{dt}", name=f"outb{dt}")
                         for dt in range(DT2)]
                tmpf = xp.tile([P, NPG], F32, tag="tmpf", name="tmpf")

                def emit_stats(xt, dt, c):
                    # raw moments; dt0 on ACT (Identity/Square are in every
                    # activation table - no table thrash), dt1 on DVE
                    scr = tmpf if dt else outf[0]
                    if dt == 0:
                        nc.scalar.activation(out=scr[:], in_=xt[dt][:],
                                             func=AF.Identity,
                                             accum_out=stats[:, c:c + 1])
                        nc.scalar.activation(out=scr[:], in_=xt[dt][:],
                                             func=AF.Square,
                                             accum_out=stats[:, c + 1:c + 2])
                    else:
                        nc.vector.scalar_tensor_tensor(
                            out=scr[:], in0=xt[dt][:], scalar=0.0, in1=xt[dt][:],
                            op0=ALU.mult, op1=ALU.add,
                            accum_out=stats[:, c:c + 1])
                        nc.vector.scalar_tensor_tensor(
                            out=scr[:], in0=xt[dt][:], scalar=1.0, in1=xt[dt][:],
                            op0=ALU.mult, op1=ALU.mult,
                            accum_out=stats[:, c + 1:c + 2])

                for dt in range(DT2):
                    emit_stats(x2T, dt, 4 + dt * 2)

                # ---- SAGE local transform -> x1 (needs RS result) ----
                x1T = [xp.tile([P, NPG], F32, tag=f"x1T{dt}", name=f"x1T{dt}")
                       for dt in range(DT2)]
                for dt in range(DT2):
                    def drain_x1(ps, s0, w, dt=dt):
                        nc.vector.scalar_tensor_tensor(
                            out=x1T[dt][:, s0:s0 + w], in0=ps[:, 0:w],
                            scalar=bias_ap(l, 0, dt),
                            in1=hT_f[dt][:, s0:s0 + w],
                            op0=ALU.add, op1=ALU.add)
                    # wr@h terms first: they only need h, so the PE can start
                    # while the ReduceScatter readback is still landing
                    lhs = ([wrT[l][:, kt, dt * P:(dt + 1) * P] for kt in range(DT2)]
                           + [wlT[l][:, kt, dt * P:(dt + 1) * P] for kt in range(DT2)])
                    rhs = [hT_b[kt][:] for kt in range(DT2)] \
                        + [agg_sb[kt][:] for kt in range(DT2)]
                    mm_slices(lhs, rhs, NPG, drain_x1)

                # ---- BN stats for n1 (x1), then the joint AllGather ----
                for dt in range(DT2):
                    emit_stats(x1T, dt, dt * 2)
                cc_in = dram.tile([P, 8], F32, tag="r1in", name="r1in")
                cc_out = dram.tile([NCORES, P, 8], F32, tag="r1out", name="r1out",
                                   addr_space="Shared")
                nc.sync.dma_start(out=cc_in[:], in_=stats[:])
                nc.gpsimd.collective_compute(
                    "AllGather", ALU.bypass, replica_groups=RG,
                    ins=[cc_in[:].opt()], outs=[cc_out[:].opt()],
                )
                gsum = small.tile([P, NCORES, 8], F32, tag="gsum", name="gsum")
                nc.sync.dma_start(out=gsum[:],
                                  in_=cc_out[:].rearrange("r p s -> p r s"))
                nc.vector.tensor_add(out=gsum[:, 0:4, :], in0=gsum[:, 0:4, :],
                                     in1=gsum[:, 4:8, :])
                nc.vector.tensor_add(out=gsum[:, 0:2, :], in0=gsum[:, 0:2, :],
                                     in1=gsum[:, 2:4, :])
                nc.vector.tensor_add(out=gsum[:, 0, :], in0=gsum[:, 0, :],
                                     in1=gsum[:, 1, :])
                gm = small.tile([P, 8], F32, tag="gm", name="gm")
                nc.vector.tensor_scalar(out=gm[:], in0=gsum[:, 0, :],
                                        scalar1=1.0 / N, scalar2=None,
                                        op0=ALU.mult)

                # batched scale/shift for n1 (cols 0,1) and n2 (cols 2,3), per dt
                def gap(t, off, n, stride):
                    a = t[:]
                    return bass.AP(tensor=a.tensor, offset=a.offset + off,
                                   ap=[list(a.ap[0])] + [[stride, n]])
                m4, e4 = gap(gm, 0, 4, 2), gap(gm, 1, 4, 2)
                var4 = small.tile([P, 4], F32, tag="var4", name="var4")
                sc4 = small.tile([P, 4], F32, tag="sc4", name="sc4")
                t4 = small.tile([P, 4], F32, tag="t4", name="t4")
                nc.vector.tensor_tensor(out=var4[:], in0=m4, in1=m4, op=ALU.mult)
                nc.vector.tensor_tensor(out=var4[:], in0=e4, in1=var4[:],
                                        op=ALU.subtract)
                nc.scalar.activation(out=var4[:], in_=var4[:], func=AF.Sqrt,
                                     bias=eps_t[:])
                nc.vector.reciprocal(out=var4[:], in_=var4[:])
                # w/b for (n1,dt0),(n1,dt1),(n2,dt0),(n2,dt1): nrm idx 0,2 / 1,3
                nv = nrmv[l][:]
                w4 = bass.AP(tensor=nv.tensor, offset=nv.offset,
                             ap=[list(nv.ap[0])] + [[4, 2], [1, 2]])
                b4 = bass.AP(tensor=nv.tensor, offset=nv.offset + 2,
                             ap=[list(nv.ap[0])] + [[4, 2], [1, 2]])
                nc.vector.tensor_tensor(out=sc4[:], in0=var4[:],
                                        in1=w4, op=ALU.mult)
                nc.vector.tensor_tensor(out=t4[:], in0=m4, in1=sc4[:], op=ALU.mult)
                nc.vector.tensor_tensor(out=t4[:], in0=b4, in1=t4[:],
                                        op=ALU.subtract)
                tc2 = small.tile([P, 2], F32, tag="tc2", name="tc2")
                nc.vector.tensor_add(out=tc2[:], in0=t4[:, 0:2], in1=t4[:, 2:4])

                # ---- out = n1(x1) + n2(x2) ----
                for dt in range(DT2):
                    for s in range(2):
                        sl = slice(s * 512, (s + 1) * 512)
                        nc.scalar.activation(out=outf[dt][:, sl], in_=x1T[dt][:, sl],
                                             func=AF.Identity,
                                             scale=sc4[:, dt:dt + 1],
                                             bias=tc2[:, dt:dt + 1])
                        nc.vector.scalar_tensor_tensor(
                            out=out_b[dt][:, sl], in0=x2T[dt][:, sl],
                            scalar=sc4[:, 2 + dt:3 + dt], in1=outf[dt][:, sl],
                            op0=ALU.mult, op1=ALU.add)

                # ---- MLP residual ----
                relu1 = [qkp.tile([P, NPG], BF16, tag=f"relu1{ft}", name=f"relu1{ft}")
                         for ft in range(FT4)]
                for ft in range(FT4):
                    def drain_r(ps, s0, w, ft=ft):
                        nc.scalar.activation(out=relu1[ft][:, s0:s0 + w],
                                             in_=ps[:, 0:w], func=AF.Relu,
                                             bias=b1v[l][:, ft:ft + 1])
                    mm_slices(
                        [w1T[l][:, kt, ft * P:(ft + 1) * P] for kt in range(DT2)],
                        [out_b[kt][:] for kt in range(DT2)], NPG, drain_r)
                out2 = [xp.tile([P, NPG], F32, tag=f"out2{dt}", name=f"out2{dt}")
                        for dt in range(DT2)]
                stats3 = small.tile([P, 4], F32, tag="stats3", name="stats3")
                for dt in range(DT2):
                    def drain_m(ps, s0, w, dt=dt):
                        nc.vector.scalar_tensor_tensor(
                            out=out2[dt][:, s0:s0 + w], in0=ps[:, 0:w],
                            scalar=bias_ap(l, 4, dt),
                            in1=out_b[dt][:, s0:s0 + w],
                            op0=ALU.add, op1=ALU.add)
                    mm_slices(
                        [w2T[l][:, kt, dt * P:(dt + 1) * P] for kt in range(FT4)],
                        [relu1[kt][:] for kt in range(FT4)], NPG, drain_m)
                for dt in range(DT2):
                    scr = tmpf if dt else outf[0]
                    c = dt * 2
                    if dt == 0:
                        nc.scalar.activation(out=scr[:], in_=out2[dt][:],
                                             func=AF.Identity,
                                             accum_out=stats3[:, c:c + 1])
                        nc.scalar.activation(out=scr[:], in_=out2[dt][:],
                                             func=AF.Square,
                                             accum_out=stats3[:, c + 1:c + 2])
                    else:
                        nc.vector.scalar_tensor_tensor(
                            out=scr[:], in0=out2[dt][:], scalar=0.0,
                            in1=out2[dt][:], op0=ALU.mult, op1=ALU.add,
                            accum_out=stats3[:, c:c + 1])
                        nc.vector.scalar_tensor_tensor(
                            out=scr[:], in0=out2[dt][:], scalar=1.0,
                            in1=out2[dt][:], op0=ALU.mult, op1=ALU.mult,
                            accum_out=stats3[:, c + 1:c + 2])
                cc3_in = dram.tile([P, 4], F32, tag="r2in", name="r2in")
                cc3_out = dram.tile([NCORES, P, 4], F32, tag="r2out", name="r2out",
                                    addr_space="Shared")
                nc.sync.dma_start(out=cc3_in[:], in_=stats3[:])
                nc.gpsimd.collective_compute(
                    "AllGather", ALU.bypass, replica_groups=RG,
                    ins=[cc3_in[:].opt()], outs=[cc3_out[:].opt()],
                )
                gsum3 = small.tile([P, NCORES, 4], F32, tag="gsum3", name="gsum3")
                nc.sync.dma_start(out=gsum3[:],
                                  in_=cc3_out[:].rearrange("r p s -> p r s"))
                nc.vector.tensor_add(out=gsum3[:, 0:4, :], in0=gsum3[:, 0:4, :],
                                     in1=gsum3[:, 4:8, :])
                nc.vector.tensor_add(out=gsum3[:, 0:2, :], in0=gsum3[:, 0:2, :],
                                     in1=gsum3[:, 2:4, :])
                nc.vector.tensor_add(out=gsum3[:, 0, :], in0=gsum3[:, 0, :],
                                     in1=gsum3[:, 1, :])
                g3 = small.tile([P, 4], F32, tag="g3", name="g3")
                nc.vector.tensor_scalar(out=g3[:], in0=gsum3[:, 0, :],
                                        scalar1=1.0 / N, scalar2=None,
                                        op0=ALU.mult)
                # batched over dt: m3 = cols 0,2 ; e3 = cols 1,3
                m2_, e2_ = gap(g3, 0, 2, 2), gap(g3, 1, 2, 2)
                v2 = small.tile([P, 2], F32, tag="v2", name="v2")
                r2 = small.tile([P, 2], F32, tag="r2", name="r2")
                al2 = small.tile([P, 2], F32, tag="al2", name="al2")
                be2 = small.tile([P, 2], F32, tag="be2", name="be2")
                nc.vector.tensor_tensor(out=v2[:], in0=m2_, in1=m2_, op=ALU.mult)
                nc.vector.tensor_tensor(out=v2[:], in0=e2_, in1=v2[:],
                                        op=ALU.subtract)
                nc.scalar.activation(out=r2[:], in_=v2[:], func=AF.Sqrt,
                                     bias=eps_t[:])
                nc.vector.reciprocal(out=r2[:], in_=r2[:])
                w3_ = bass.AP(tensor=nv.tensor, offset=nv.offset + 4 * 2,
                              ap=[list(nv.ap[0])] + [[1, 2]])   # n3_w per dt
                bw_ = bass.AP(tensor=nv.tensor, offset=nv.offset + 6 * 2,
                              ap=[list(nv.ap[0])] + [[1, 2]])   # bn_w per dt
                bb_ = bass.AP(tensor=nv.tensor, offset=nv.offset + 7 * 2,
                              ap=[list(nv.ap[0])] + [[1, 2]])   # bn_b per dt
                # al = w3*r3; rbn = rsqrt(al^2*v3+eps); al = al*rbn*bw; be = bb-m3*al
                nc.vector.tensor_tensor(out=al2[:], in0=w3_, in1=r2[:], op=ALU.mult)
                nc.vector.tensor_tensor(out=be2[:], in0=al2[:], in1=al2[:],
                                        op=ALU.mult)
                nc.vector.tensor_tensor(out=be2[:], in0=be2[:], in1=v2[:],
                                        op=ALU.mult)
                nc.scalar.activation(out=be2[:], in_=be2[:], func=AF.Sqrt,
                                     bias=eps_t[:])
                nc.vector.reciprocal(out=be2[:], in_=be2[:])
                nc.vector.tensor_tensor(out=al2[:], in0=al2[:], in1=be2[:],
                                        op=ALU.mult)
                nc.vector.tensor_tensor(out=al2[:], in0=al2[:], in1=bw_, op=ALU.mult)
                nc.vector.tensor_tensor(out=be2[:], in0=m2_, in1=al2[:], op=ALU.mult)
                nc.vector.tensor_tensor(out=be2[:], in0=bb_, in1=be2[:],
                                        op=ALU.subtract)
                hT_f_new = [featp.tile([P, NPG], F32, tag=f"hTf{dt}", name=f"hTf{dt}")
                            for dt in range(DT2)]
                hT_b_new = [featp.tile([P, NPG], BF16, tag=f"hTb{dt}",
                                       name=f"hTb{dt}") for dt in range(DT2)]
                for dt in range(DT2):
                    for s in range(2):
                        sl = slice(s * 512, (s + 1) * 512)
                        nc.scalar.activation(out=tmpf[:, sl], in_=out2[dt][:, sl],
                                             func=AF.Relu,
                                             scale=al2[:, dt:dt + 1],
                                             bias=be2[:, dt:dt + 1])
                        nc.vector.tensor_add(out=hT_f_new[dt][:, sl],
                                             in0=hT_f[dt][:, sl], in1=tmpf[:, sl])
                    if l < L - 1:  # bf16 h only feeds next layer's matmuls
                        nc.gpsimd.tensor_copy(out=hT_b_new[dt][:],
                                              in_=hT_f_new[dt][:])
                hT_f, hT_b = hT_f_new, hT_b_new

            # ---------------- pool + head ----------------
            pooled = small.tile([P, DT2], F32, tag="pooled", name="pooled")
            pooled_b = small.tile([P, DT2], BF16, tag="pooledb", name="pooledb")
            for dt in range(DT2):
                nc.vector.tensor_reduce(out=pooled[:, dt:dt + 1], in_=hT_f[dt][:],
                                        axis=mybir.AxisListType.X, op=ALU.add)
            nc.scalar.activation(out=pooled_b[:], in_=pooled[:], func=AF.Identity,
                                 scale=1.0 / NPG)
            ps_y = psA.tile([P, 512], F32, space="PSUM", tag="a", name="a")
            for dt in range(DT2):
                nc.tensor.matmul(out=ps_y[0:OUT_D, 0:1],
                                 lhsT=w_outT[:, dt, :],
                                 rhs=pooled_b[:, dt:dt + 1],
                                 start=(dt == 0), stop=(dt == DT2 - 1))
            y_sb = small.tile([OUT_D, 1], F32, tag="ysb", name="ysb")
            nc.scalar.activation(out=y_sb[:], in_=ps_y[0:OUT_D, 0:1],
                                 func=AF.Identity, bias=boutv[:])
            nc.sync.dma_start(out=y_out[:, :], in_=y_sb[:])

    return nc


# ---------------------------------------------------------------------------
# Host-side: shard inputs, run, gather
# ---------------------------------------------------------------------------
def prep_inputs(x, edge_index, batch, w_in, b_in, sage_wl, sage_bl, sage_wr,
                attn_iw, attn_ib, attn_ow, attn_ob, n1_w, n1_b, n2_w, n2_b,
                n3_w, n3_b, mlp_w1, mlp_b1, mlp_w2, mlp_b2, bn_w, bn_b,
                w_out, b_out):
    bf = ml_dtypes.bfloat16
    f8 = ml_dtypes.float8_e4m3
    x = np.asarray(x, np.float32)
    ei = np.asarray(edge_index)
    src, dst = np.asarray(ei[0], np.int64), np.asarray(ei[1], np.int64)
    deg = np.bincount(dst, minlength=N).astype(np.float32)
    inv_deg = 1.0 / np.clip(deg, 1.0, None)

    def t32(a):
        return np.ascontiguousarray(np.asarray(a, np.float32))

    def packT(w_l):  # [out, in] -> [K=in/P, P, out] (transposed, packed)
        wt = t32(w_l).T  # [in, out]
        return wt.reshape(wt.shape[0] // P, P, wt.shape[1])

    shared = {
        "w_inT": t32(w_in).T.astype(bf),                       # [128, 256]
        "w_outT": packT(w_out).astype(bf),                     # [2, 128, 64]
        "wlT": np.stack([packT(sage_wl[l]) for l in range(L)]).astype(bf),
        "wrT": np.stack([packT(sage_wr[l]) for l in range(L)]).astype(bf),
        "wqT": np.stack([packT(attn_iw[l][0:D]) for l in range(L)]).astype(bf),
        "wkT": np.stack([packT(attn_iw[l][D:2 * D]) for l in range(L)]).astype(bf),
        "wvT": np.stack([packT(attn_iw[l][2 * D:3 * D]) for l in range(L)]).astype(bf),
        "owT": np.stack([packT(attn_ow[l]) for l in range(L)]).astype(bf),
        "w1T": np.stack([packT(mlp_w1[l]) for l in range(L)]).astype(bf),
        "w2T": np.stack([packT(mlp_w2[l]) for l in range(L)]).astype(bf),
        "vbr": np.stack([t32(attn_ib[l][2 * D:3 * D])[None, :]
                         for l in range(L)]).astype(bf),
        "b1v": np.stack([t32(mlp_b1[l]).reshape(FT4, P) for l in range(L)]),
        "boutv": t32(b_out)[:, None],
    }
    biasv = np.zeros((L, 8, DT2, P), np.float32)
    nrmv = np.zeros((L, 8, DT2, P), np.float32)
    for l in range(L):
        biasv[l, 0] = t32(sage_bl[l]).reshape(DT2, P)
        biasv[l, 1] = t32(attn_ib[l][0:D]).reshape(DT2, P)
        biasv[l, 2] = t32(attn_ib[l][D:2 * D]).reshape(DT2, P)
        biasv[l, 3] = t32(attn_ob[l]).reshape(DT2, P)
        biasv[l, 4] = t32(mlp_b2[l]).reshape(DT2, P)
        if l == 0:
            biasv[l, 5] = t32(b_in).reshape(DT2, P)
        nrmv[l, 0] = t32(n1_w[l]).reshape(DT2, P)
        nrmv[l, 1] = t32(n1_b[l]).reshape(DT2, P)
        nrmv[l, 2] = t32(n2_w[l]).reshape(DT2, P)
        nrmv[l, 3] = t32(n2_b[l]).reshape(DT2, P)
        nrmv[l, 4] = t32(n3_w[l]).reshape(DT2, P)
        nrmv[l, 5] = t32(n3_b[l]).reshape(DT2, P)
        nrmv[l, 6] = t32(bn_w[l]).reshape(DT2, P)
        nrmv[l, 7] = t32(bn_b[l]).reshape(DT2, P)
    shared["biasv"] = biasv
    shared["nrmv"] = nrmv

    in_maps = []
    for c in range(NCORES):
        lo, hi = c * NPG, (c + 1) * NPG
        sel = (src >= lo) & (src < hi)
        s_c, d_c = src[sel] - lo, dst[sel]
        at = np.zeros(NPG * N, np.float32)
        np.add.at(at, s_c * N + d_c, 1.0)
        m = dict(shared)
        m["xT"] = np.ascontiguousarray(x[lo:hi].T).astype(bf)
        m["at"] = at.reshape(NPG, N).astype(f8)
        m["invd"] = inv_deg[None, :].astype(bf)
        in_maps.append(m)
    return in_maps


_NC_CACHE = {}


def get_nc():
    if "nc" not in _NC_CACHE:
        _NC_CACHE["nc"] = build_kernel()
    return _NC_CACHE["nc"]


def kernel(**inputs):
    in_maps = prep_inputs(**inputs)
    nc = get_nc()
    res = run_bass_kernel_spmd(nc, in_maps, list(range(NCORES)))
    out = np.stack([res.results[c]["y"][:, 0] for c in range(NCORES)])
    return out.astype(np.float32)


if __name__ == "__main__":
    rng = np.random.default_rng(0)
    ins = dict(
        x=rng.standard_normal((N, IN_C), dtype=np.float32),
        edge_index=rng.integers(0, N, (2, E)),
        batch=np.arange(N, dtype=np.int32) // NPG,
        w_in=rng.standard_normal((D, IN_C), dtype=np.float32) * 0.05,
        b_in=rng.standard_normal(D, dtype=np.float32) * 0.05,
        sage_wl=rng.standard_normal((L, D, D), dtype=np.float32) * 0.05,
        sage_bl=rng.standard_normal((L, D), dtype=np.float32) * 0.05,
        sage_wr=rng.standard_normal((L, D, D), dtype=np.float32) * 0.05,
        attn_iw=rng.standard_normal((L, 3 * D, D), dtype=np.float32) * 0.05,
        attn_ib=rng.standard_normal((L, 3 * D), dtype=np.float32) * 0.05,
        attn_ow=rng.standard_normal((L, D, D), dtype=np.float32) * 0.05,
        attn_ob=rng.standard_normal((L, D), dtype=np.float32) * 0.05,
        n1_w=np.ones((L, D), np.float32), n1_b=np.zeros((L, D), np.float32),
        n2_w=np.ones((L, D), np.float32), n2_b=np.zeros((L, D), np.float32),
        n3_w=np.ones((L, D), np.float32), n3_b=np.zeros((L, D), np.float32),
        mlp_w1=rng.standard_normal((L, DFF, D), dtype=np.float32) * 0.05,
        mlp_b1=rng.standard_normal((L, DFF), dtype=np.float32) * 0.05,
        mlp_w2=rng.standard_normal((L, D, DFF), dtype=np.float32) * 0.05,
        mlp_b2=rng.standard_normal((L, D), dtype=np.float32) * 0.05,
        bn_w=np.ones((L, D), np.float32), bn_b=np.zeros((L, D), np.float32),
        w_out=rng.standard_normal((OUT_D, D), dtype=np.float32) * 0.05,
        b_out=rng.standard_normal(OUT_D, dtype=np.float32) * 0.05,
    )
    y = kernel(**ins)
    print("y shape:", y.shape, "finite:", np.isfinite(y).all())


# revision 53
# speedup vs baseline: 1.0465x; 1.0055x over previous
"""GPS (GraphGPS) forward pass on 8 Trainium2 NeuronCores.

Model (from the reference): 2 layers of
  SAGEConv(mean aggr) + residual + BN  ||  per-graph dense MHA + residual + BN
  -> sum branches -> MLP residual -> BN -> outer BN + relu + residual
then per-graph mean pool + linear head.

Sharding: one graph (1024 nodes) per core. The SAGE neighbor aggregation is
computed ReduceScatter-style: each core multiplies its LOCAL node features
h_c [1024, 256] against its src-slice of the dense edge-count matrix
A_c [1024 src x 8192 dst] (fp8 counts, exact small ints), producing partial
aggregates for ALL destinations; a ReduceScatter(add) then hands every core
the summed aggregate rows for its own 1024 destinations, which are scaled by
1/deg locally. This needs no AllGather of features at all. BatchNorm batch
stats are exchanged with small AllGathers (cheaper than AllReduce here) and
summed locally.

Device layout: features kept transposed (hT = [256 dims x 1024 nodes], dims
on partitions) so BN stats/apply are per-partition ops; h natural
([node, dim], from 16 PE transposes per layer) feeds the SAGE matmul as lhsT.
Attention: scores^T [keys, q] per (head, key-tile); exp on ACT; PV contracts
over keys with the 33-wide (V ++ ones) natural V so output lands natural
[q, d] with the softmax denominator on the same partition as its query row
(per-partition normalize), then 16 PE transposes take O back to d-major for
the out-projection.
"""
import numpy as np
import ml_dtypes

import concourse.bass as bass
import concourse.mybir as mybir
import concourse.tile as tile
from concourse.bass_utils import run_bass_kernel_spmd
from concourse.vector_clock import ScopedClock
from concourse.masks import make_identity

# ---------------------------------------------------------------------------
# Walrus workaround: this toolchain rejects >1 sync-wait command per
# instruction. Hoist excess waits onto same-engine NoOps / extra drains.
# ---------------------------------------------------------------------------
_MAX_WAITS = 1


def _split_waits_in_ordered(nc, ordered):
    for bb_name, insts in ordered.items():
        new_list = []
        for inst in insts:
            si = getattr(inst, "sync_info", None)
            if si is not None and si.on_wait and len(si.on_wait) > _MAX_WAITS:
                waits = list(si.on_wait)
                keep = waits[-_MAX_WAITS:]
                for w in waits[:-_MAX_WAITS]:
                    nop = mybir.InstNoOp(
                        name=nc.get_next_instruction_name(),
                        engine=inst.engine,
                        ins=[],
                        outs=[],
                        sync_info=mybir.SyncInfo(on_wait=[w], on_update=[]),
                    )
                    nop.debug = inst.debug
                    new_list.append(nop)
                si.on_wait[:] = keep
            new_list.append(inst)
        insts[:] = new_list


_orig_lower = tile.TileContext._lower_ordered_insts


def _patched_lower_ordered_insts(self, ordered):
    _split_waits_in_ordered(self.nc, ordered)
    return _orig_lower(self, ordered)


def _patched_drain_and_barrier(self, tick_clock, wait_clock):
    drain_inst = self.nc.sync.drain()
    wait_clock.add_sem_waits(drain_inst.ins, ScopedClock({None: tick_clock.global_clock}))
    si = drain_inst.ins.sync_info
    waits = list(si.on_wait) if si is not None else []
    if len(waits) > _MAX_WAITS:
        si.on_wait[:] = waits[:_MAX_WAITS]
        for w in waits[_MAX_WAITS:]:
            d2 = self.nc.sync.drain()
            d2.ins.sync_info = mybir.SyncInfo(on_wait=[w], on_update=[])
    self.nc.all_engine_barrier()
    assert self.sems is not None
    popped = self.nc._tile_sem_poison_stack.pop()
    assert popped is self._sem_poison
    self.nc.clear_and_free_semaphores(list(self.sems.allocated().values()))
    self.nc.all_engine_barrier()


tile.TileContext._lower_ordered_insts = _patched_lower_ordered_insts
tile.TileContext._drain_and_barrier = _patched_drain_and_barrier

# ---------------------------------------------------------------------------
# Problem constants (hardcoded per the task contract)
# ---------------------------------------------------------------------------
N, B, NPG = 8192, 8, 1024
D, H, DH, L = 256, 8, 32, 2
IN_C, OUT_D, E, DFF = 128, 64, 262144, 512
EPS = 1e-5
NCORES = 8
P = 128          # SBUF partitions
DT2 = D // P     # 2 dim tiles of 128
FT4 = DFF // P   # 4 ff tiles
NT8 = NPG // P   # 8 local node tiles
CH = 16          # dst chunks for the SAGE partial matmul
CHW = N // CH    # 512 dst per chunk
F32 = mybir.dt.float32
BF16 = mybir.dt.bfloat16
FP8 = mybir.dt.float8e4
AF = mybir.ActivationFunctionType
ALU = mybir.AluOpType
RG = [list(range(NCORES))]


def build_kernel():
    nc = bass.Bass()

    # ---- I/O declarations ----
    xT_in = nc.dram_tensor("xT", [P, NPG], BF16, kind="ExternalInput")
    at_in = nc.dram_tensor("at", [NPG, N], FP8, kind="ExternalInput")
    invd_in = nc.dram_tensor("invd", [1, N], BF16, kind="ExternalInput")
    # per-layer weights, host-transposed; leading dims packed for [128, ...] SBUF tiles
    wlT_in = nc.dram_tensor("wlT", [L, DT2, P, D], BF16, kind="ExternalInput")
    wrT_in = nc.dram_tensor("wrT", [L, DT2, P, D], BF16, kind="ExternalInput")
    wqT_in = nc.dram_tensor("wqT", [L, DT2, P, D], BF16, kind="ExternalInput")
    wkT_in = nc.dram_tensor("wkT", [L, DT2, P, D], BF16, kind="ExternalInput")
    wvT_in = nc.dram_tensor("wvT", [L, DT2, P, D], BF16, kind="ExternalInput")
    owT_in = nc.dram_tensor("owT", [L, DT2, P, D], BF16, kind="ExternalInput")
    w1T_in = nc.dram_tensor("w1T", [L, DT2, P, DFF], BF16, kind="ExternalInput")
    w2T_in = nc.dram_tensor("w2T", [L, FT4, P, D], BF16, kind="ExternalInput")
    w_inT_in = nc.dram_tensor("w_inT", [IN_C, D], BF16, kind="ExternalInput")
    w_outT_in = nc.dram_tensor("w_outT", [DT2, P, OUT_D], BF16, kind="ExternalInput")
    # biases / norm params, fp32; [idx, dt, p] so device holds [p, idx, dt]
    bias_in = nc.dram_tensor("biasv", [L, 8, DT2, P], F32, kind="ExternalInput")
    #   biasv[l]: 0=sage_b 1=qb 2=kb 3=ob 4=b2 5=b_in(l0) 6,7 spare
    b1_in = nc.dram_tensor("b1v", [L, FT4, P], F32, kind="ExternalInput")
    nrm_in = nc.dram_tensor("nrmv", [L, 8, DT2, P], F32, kind="ExternalInput")
    #   nrmv[l]: 0=n1_w 1=n1_b 2=n2_w 3=n2_b 4=n3_w 5=n3_b 6=bn_w 7=bn_b
    vb_in = nc.dram_tensor("vbr", [L, 1, D], BF16, kind="ExternalInput")
    bout_in = nc.dram_tensor("boutv", [OUT_D, 1], F32, kind="ExternalInput")

    y_out = nc.dram_tensor("y", [OUT_D, 1], F32, kind="ExternalOutput")

    with tile.TileContext(nc) as tc:
        with (
            tc.tile_pool(name="wpool", bufs=1) as wpool,      # persistent weights
            tc.tile_pool(name="featp", bufs=2) as featp,      # hT (old/new rotate)
            tc.tile_pool(name="natp", bufs=1) as natp,        # h natural + agg
            tc.tile_pool(name="qkp", bufs=1) as qkp,          # Q/K/V per layer
            tc.tile_pool(name="expp", bufs=16) as expp,       # exp(score) tiles
            tc.tile_pool(name="onp", bufs=1) as onp,          # O_nat / OT
            tc.tile_pool(name="xp", bufs=1) as xp,            # x1/x2/out/out2
            tc.tile_pool(name="stg", bufs=2) as stg,          # RS staging chunks
            tc.tile_pool(name="small", bufs=4) as small,      # stats etc
            tc.tile_pool(name="atp", bufs=4) as atp,          # A chunk stream
            tc.tile_pool(name="psA", bufs=2, space="PSUM") as psA,   # 2 banks
            tc.tile_pool(name="psS", bufs=2, space="PSUM") as psS,   # 4 banks
            tc.tile_pool(name="psV", bufs=1, space="PSUM") as psV,   # 1 bank
            tc.tile_pool(name="dram", bufs=2, space="DRAM") as dram,
        ):
            assert nc.vector.BN_STATS_FMAX >= 512

            # ---------------- load weights ----------------
            def load_w(shape, src_ap, name, dtype=BF16, pool=wpool):
                t = pool.tile(shape, dtype, tag=name, name=name)
                nc.sync.dma_start(out=t[:], in_=src_ap)
                return t

            # order matters: in_proj inputs + small params first so the first
            # matmuls aren't queued behind the big weight streams
            xT = load_w([P, NPG], xT_in[:, :], "xTw")
            w_inT = load_w([IN_C, D], w_inT_in[:, :], "w_inTw")
            biasv = [load_w([P, 8, DT2], bias_in[l].rearrange("i t p -> p i t"),
                            f"biasw{l}", F32) for l in range(L)]
            nrmv = [load_w([P, 8, DT2], nrm_in[l].rearrange("i t p -> p i t"),
                           f"nrmw{l}", F32) for l in range(L)]
            b1v = [load_w([P, FT4], b1_in[l].rearrange("t p -> p t"),
                          f"b1w{l}", F32) for l in range(L)]
            vbr = [load_w([1, D], vb_in[l], f"vbrw{l}") for l in range(L)]
            boutv = load_w([OUT_D, 1], bout_in[:, :], "boutw", F32)

            def load_packed(src, free, nm):
                # src [L, K, P, free] -> per-layer tiles [P, K, free]
                return [load_w([P, src.shape[1], free],
                               src[l].rearrange("k p f -> p k f"), f"{nm}{l}")
                        for l in range(L)]

            wqT = load_packed(wqT_in, D, "wqTw")
            wkT = load_packed(wkT_in, D, "wkTw")
            wvT = load_packed(wvT_in, D, "wvTw")
            owT = load_packed(owT_in, D, "owTw")
            wlT = load_packed(wlT_in, D, "wlTw")
            wrT = load_packed(wrT_in, D, "wrTw")
            w1T = load_packed(w1T_in, DFF, "w1Tw")
            w2T = load_packed(w2T_in, D, "w2Tw")
            w_outT = load_w([P, DT2, OUT_D], w_outT_in[:].rearrange("t p o -> p t o"),
                            "w_outTw")

            # global inv_deg, broadcast to all partitions: folded into the
            # SAGE partial drains (pre-ReduceScatter), so the RS result is
            # the finished mean aggregation
            invd_bc = wpool.tile([P, N], BF16, tag="invdbc", name="invdbc")
            iv_ap = invd_in[:, :]
            nc.sync.dma_start(
                out=invd_bc[:],
                in_=bass.AP(tensor=iv_ap.tensor, offset=iv_ap.offset,
                            ap=[[0, P]] + list(iv_ap.ap[1:])),
            )

            ones_row = wpool.tile([1, P], BF16)
            nc.vector.memset(ones_row[:], 1.0)
            eps_t = wpool.tile([P, 1], F32)
            nc.vector.memset(eps_t[:], EPS)
            ident = wpool.tile([P, P], F32)
            make_identity(nc, ident[:])

            def bias_ap(l, idx, dt):
                return biasv[l][:, idx, dt:dt + 1]

            def nrm_ap(l, idx, dt):
                return nrmv[l][:, idx, dt:dt + 1]

            # generic matmul into psA 512-slices with per-slice drain callback
            def mm_slices(lhsT_aps, rhs_aps, nfree, drain, slice_w=512):
                for s0 in range(0, nfree, slice_w):
                    w = min(slice_w, nfree - s0)
                    ps = psA.tile([P, 512], F32, space="PSUM", tag="a", name="a")
                    nk = len(lhsT_aps)
                    for k in range(nk):
                        nc.tensor.matmul(
                            out=ps[:, 0:w], lhsT=lhsT_aps[k],
                            rhs=rhs_aps[k][:, s0:s0 + w],
                            start=(k == 0), stop=(k == nk - 1),
                        )
                    drain(ps, s0, w)

            # ---------------- in_proj ----------------
            hT_f = [featp.tile([P, NPG], F32, tag=f"hTf{dt}", name=f"hTf{dt}")
                    for dt in range(DT2)]
            hT_b = [featp.tile([P, NPG], BF16, tag=f"hTb{dt}", name=f"hTb{dt}")
                    for dt in range(DT2)]
            for dt in range(DT2):
                def drain_in(ps, s0, w, dt=dt):
                    nc.scalar.activation(out=hT_f[dt][:, s0:s0 + w], in_=ps[:, 0:w],
                                         func=AF.Identity, bias=bias_ap(0, 5, dt))
                mm_slices([w_inT[:, dt * P:(dt + 1) * P]], [xT[:]], NPG, drain_in)
                nc.gpsimd.tensor_copy(out=hT_b[dt][:], in_=hT_f[dt][:])

            # ---------------- layers ----------------
            for l in range(L):
                # ---- A chunk prefetch (first 4; rest issued inside interleave)
                at_tiles = [None] * CH

                def fetch_chunk(c):
                    t = atp.tile([P, NT8, CHW], FP8, tag="att", name="att")
                    nc.sync.dma_start(
                        out=t[:],
                        in_=at_in[:, c * CHW:(c + 1) * CHW]
                        .rearrange("(kt p) f -> p kt f", p=P))
                    at_tiles[c] = t

                for c in range(4):
                    fetch_chunk(c)

                # ---- h natural via PE transposes (psV ring as scratch)
                # fp8 so the SAGE matmul can run in DoubleRow (2x) perf mode
                h_nat = natp.tile([P, NT8, D], FP8, tag="hnat", name="hnat")
                for nt in range(NT8):
                    for dt in range(DT2):
                        pst = psS.tile([P, NPG], F32, space="PSUM", tag="s", name="s")
                        nc.tensor.transpose(
                            out=pst[:, 0:P],
                            in_=hT_f[dt][:, nt * P:(nt + 1) * P],
                            identity=ident[:],
                        )
                        nc.vector.tensor_copy(out=h_nat[:, nt, dt * P:(dt + 1) * P],
                                              in_=pst[:, 0:P])

                # ---- Q/K projections (d-major) ----
                QT = [qkp.tile([P, NPG], BF16, tag=f"QT{dt}", name=f"QT{dt}")
                      for dt in range(DT2)]
                KT = [qkp.tile([P, NPG], BF16, tag=f"KT{dt}", name=f"KT{dt}")
                      for dt in range(DT2)]
                for dst, w_t, b_idx in ((QT, wqT[l], 1), (KT, wkT[l], 2)):
                    for dt in range(DT2):
                        def drain_qk(ps, s0, w, dst=dst, dt=dt, b_idx=b_idx):
                            nc.vector.tensor_scalar(
                                out=dst[dt][:, s0:s0 + w], in0=ps[:, 0:w],
                                scalar1=bias_ap(l, b_idx, dt), scalar2=None,
                                op0=ALU.add)
                        mm_slices(
                            [w_t[:, kt, dt * P:(dt + 1) * P] for kt in range(DT2)],
                            [hT_b[kt][:] for kt in range(DT2)], NPG, drain_qk)
                # stage head-3 rows (base partition 96 not addressable by PE lhsT)
                q_stg = [qkp.tile([DH, NPG], BF16, tag=f"qstg{dt}", name=f"qstg{dt}")
                         for dt in range(DT2)]
                k_stg = [qkp.tile([DH, NPG], BF16, tag=f"kstg{dt}", name=f"kstg{dt}")
                         for dt in range(DT2)]
                for dt in range(DT2):
                    nc.vector.tensor_copy(out=q_stg[dt][:], in_=QT[dt][96:128, :])
                    nc.vector.tensor_copy(out=k_stg[dt][:], in_=KT[dt][96:128, :])

                # ---- V natural per node tile with ones column (emitted inside
                # head-0's score slots, using the then-idle psV bank) ----
                Vn = [qkp.tile([P, H, DH + 1], BF16, tag=f"Vn{nt}", name=f"Vn{nt}")
                      for nt in range(NT8)]

                def emit_v(nt):
                    psv = psV.tile([P, 512], F32, space="PSUM", tag="v", name="v")
                    nc.tensor.matmul(out=psv[:, 0:D], lhsT=ones_row[:],
                                     rhs=vbr[l][:], start=True, stop=False)
                    for kt in range(DT2):
                        nc.tensor.matmul(
                            out=psv[:, 0:D],
                            lhsT=hT_b[kt][:, nt * P:(nt + 1) * P],
                            rhs=wvT[l][:, kt, :],
                            start=False, stop=(kt == DT2 - 1),
                        )
                    nc.vector.tensor_copy(out=Vn[nt][:, :, 0:DH], in_=psv[:, 0:D])
                    nc.vector.memset(Vn[nt][:, :, DH:DH + 1], 1.0)

                # ---- main interleave: attention scores/exp/PV + SAGE chunks ----
                scale = 1.0 / np.sqrt(DH)
                O_nat = onp.tile([P, NT8, D], F32, tag="onat", name="onat")
                agg_sb = [natp.tile([P, NPG], BF16, tag=f"aggsb{dt}",
                                    name=f"aggsb{dt}") for dt in range(DT2)]
                cc_rs_in = dram.tile([NCORES, DT2, P, NPG], BF16, tag="rsin",
                                     name="rsin")
                cc_rs_out = dram.tile([DT2, P, NPG], BF16, tag="rsout", name="rsout")

                # SAGE chunk emission state
                sage_state = {"next": 0, "mm": 0, "ps": None}

                def emit_sage_mms(n):
                    # emit up to n SAGE DoubleRow matmuls (kt pairs x dt)
                    for _ in range(n):
                        c = sage_state["next"]
                        if c >= CH:
                            return
                        if sage_state["mm"] == 0:
                            if at_tiles[c] is None:
                                fetch_chunk(c)
                            sage_state["ps"] = [
                                psA.tile([P, 512], F32, space="PSUM",
                                         tag="a", name="a")
                                for _ in range(DT2)]
                        i = sage_state["mm"]
                        j, dt = i // DT2, i % DT2
                        nc.tensor.matmul(
                            out=sage_state["ps"][dt][:],
                            lhsT=h_nat[:, 2 * j:2 * j + 2, dt * P:(dt + 1) * P],
                            rhs=at_tiles[c][:, 2 * j:2 * j + 2, :],
                            start=(j == 0), stop=(j == NT8 // 2 - 1),
                            perf_mode=mybir.MatmulPerfMode.DoubleRow,
                        )
                        sage_state["mm"] += 1
                        if sage_state["mm"] == NT8 // 2 * DT2:
                            # chunk complete: drain both dt planes + stage out
                            st = stg.tile([P, DT2, CHW], BF16, tag="stg", name="stg")
                            iv = invd_bc[:, c * CHW:(c + 1) * CHW]
                            nc.vector.tensor_tensor(out=st[:, 0, :], op=ALU.mult,
                                                    in0=sage_state["ps"][0][:], in1=iv)
                            nc.vector.tensor_tensor(out=st[:, 1, :], op=ALU.mult,
                                                    in0=sage_state["ps"][1][:], in1=iv)
                            cc, hh = c // 2, c % 2
                            nc.sync.dma_start(
                                out=cc_rs_in[cc, :, :, hh * CHW:(hh + 1) * CHW]
                                .rearrange("t p f -> p t f"),
                                in_=st[:])
                            if c + 4 < CH:
                                fetch_chunk(c + 4)
                            sage_state["next"] = c + 1
                            sage_state["mm"] = 0

                def emit_pv_group(hp, exp_p, pv, qt):
                    for kt in range(NT8):
                        nc.tensor.matmul(
                            out=pv[:, qt * 64:qt * 64 + DH + 1],
                            lhsT=exp_p[kt][:, qt * P:(qt + 1) * P],
                            rhs=Vn[kt][:, hp, :],
                            start=(kt == 0), stop=(kt == NT8 - 1),
                        )

                def emit_pv_norm(hp, pv):
                    # batched reciprocal of the 8 denominators (col 32+64j)
                    pv_ap = pv[:]
                    den = bass.AP(tensor=pv_ap.tensor, offset=pv_ap.offset + DH,
                                  ap=[list(pv_ap.ap[0])] + [[64, NT8]])
                    rs_h = onp.tile([P, NT8], F32, tag=f"rs{hp % 2}",
                                    name=f"rs{hp % 2}")
                    nc.vector.reciprocal(out=rs_h[:], in_=den)
                    for qt in range(NT8):
                        nc.vector.tensor_scalar(
                            out=O_nat[:, qt, hp * DH:(hp + 1) * DH],
                            in0=pv[:, qt * 64:qt * 64 + DH],
                            scalar1=rs_h[:, qt:qt + 1], scalar2=None,
                            op0=ALU.mult)

                # PV of head h-1 is threaded through head h's score slots so
                # the PE never lumps 64 PV matmuls at a head boundary
                expt, expt_prev, pv_prev = {}, None, None
                for h in range(H):
                    qdt, qr = h // 4, DH * (h % 4)
                    q_src = QT[qdt] if qr < 96 else q_stg[qdt]
                    k_src = KT[qdt] if qr < 96 else k_stg[qdt]
                    qb_, qe_ = (qr, qr + DH) if qr < 96 else (0, DH)
                    for kt in range(NT8):
                        et = expp.tile([P, NPG], BF16, tag="expt", name="expt")
                        ps_sc = psS.tile([P, NPG], F32, space="PSUM",
                                         tag="s", name="s")
                        for s in range(2):
                            nc.tensor.matmul(
                                out=ps_sc[:, s * 512:(s + 1) * 512],
                                lhsT=k_src[qb_:qe_, kt * P:(kt + 1) * P],
                                rhs=q_src[qb_:qe_, s * 512:(s + 1) * 512],
                                start=True, stop=True,
                            )
                            emit_sage_mms(2)
                        nc.scalar.activation(out=et[:], in_=ps_sc[:],
                                             func=AF.Exp, scale=scale)
                        if h == 0:
                            emit_v(kt)
                        else:
                            emit_pv_group(h - 1, expt_prev, pv_prev, kt)
                            if kt == NT8 - 1:
                                emit_pv_norm(h - 1, pv_prev)
                        expt[kt] = et
                    expt_prev, expt = expt, {}
                    pv_prev = psV.tile([P, 512], F32, space="PSUM",
                                       tag="v", name="v")
                    emit_sage_mms(4)
                # drain the last head's PV
                for qt in range(NT8):
                    emit_pv_group(H - 1, expt_prev, pv_prev, qt)
                emit_pv_norm(H - 1, pv_prev)

                # ---- finish any remaining SAGE work, then ReduceScatter ----
                emit_sage_mms(CH * NT8 * DT2)
                nc.gpsimd.collective_compute(
                    "ReduceScatter", ALU.add, replica_groups=RG,
                    ins=[cc_rs_in[:].opt()], outs=[cc_rs_out[:].opt()],
                )
                nc.sync.dma_start(
                    out=agg_sb[0][:, 0:NPG], in_=cc_rs_out[0, :, :])
                nc.sync.dma_start(
                    out=agg_sb[1][:, 0:NPG], in_=cc_rs_out[1, :, :])

                # ---- O transposes to d-major + out projection -> x2 ----
                OT = [onp.tile([P, NPG], BF16, tag=f"OT{dt}", name=f"OT{dt}")
                      for dt in range(DT2)]
                for qt in range(NT8):
                    for dt in range(DT2):
                        pst = psS.tile([P, NPG], F32, space="PSUM", tag="s", name="s")
                        nc.tensor.transpose(
                            out=pst[:, 0:P],
                            in_=O_nat[:, qt, dt * P:(dt + 1) * P],
                            identity=ident[:],
                        )
                        nc.vector.tensor_copy(out=OT[dt][:, qt * P:(qt + 1) * P],
                                              in_=pst[:, 0:P])

                x2T = [xp.tile([P, NPG], F32, tag=f"x2T{dt}", name=f"x2T{dt}")
                       for dt in range(DT2)]
                for dt in range(DT2):
                    def drain_o(ps, s0, w, dt=dt):
                        nc.vector.scalar_tensor_tensor(
                            out=x2T[dt][:, s0:s0 + w], in0=ps[:, 0:w],
                            scalar=bias_ap(l, 3, dt),
                            in1=hT_f[dt][:, s0:s0 + w],
                            op0=ALU.add, op1=ALU.add)
                    mm_slices(
                        [owT[l][:, kt, dt * P:(dt + 1) * P] for kt in range(DT2)],
                        [OT[kt][:] for kt in range(DT2)], NPG, drain_o)

                # x2 stats up-front: x2 is ready before the RS result lands,
                # so these ops must precede the x1 drains in queue order.
                # Raw moments (sum x, sum x^2) via stt accumulators, dt0 on
                # gpsimd / dt1 on DVE so the two halves run in parallel.
                stats = small.tile([P, 8], F32, tag="stats", name="stats")
                outf = [xp.tile([P, NPG], F32, tag=f"outf{dt}", name=f"outf{dt}")
                        for dt in range(DT2)]
                out_b = [xp.tile([P, NPG], BF16, tag=f"outb{dt}", name=f"outb{dt}")
                         for dt in range(DT2)]
                tmpf = xp.tile([P, NPG], F32, tag="tmpf", name="tmpf")

                def emit_stats(xt, dt, c):
                    # raw moments; dt0 on ACT (Identity/Square are in every
                    # activation table - no table thrash), dt1 on DVE
                    scr = tmpf if dt else outf[0]
                    if dt == 0:
                        nc.scalar.activation(out=scr[:], in_=xt[dt][:],
                                             func=AF.Identity,
                                             accum_out=stats[:, c:c + 1])
                        nc.scalar.activation(out=scr[:], in_=xt[dt][:],
                                             func=AF.Square,
                                             accum_out=stats[:, c + 1:c + 2])
                    else:
                        nc.vector.scalar_tensor_tensor(
                            out=scr[:], in0=xt[dt][:], scalar=0.0, in1=xt[dt][:],
                            op0=ALU.mult, op1=ALU.add,
                            accum_out=stats[:, c:c + 1])
                        nc.vector.scalar_tensor_tensor(
                            out=scr[:], in0=xt[dt][:], scalar=1.0, in1=xt[dt][:],
                            op0=ALU.mult, op1=ALU.mult,
                            accum_out=stats[:, c + 1:c + 2])

                for dt in range(DT2):
                    emit_stats(x2T, dt, 4 + dt * 2)

                # ---- SAGE local transform -> x1 (needs RS result) ----
                x1T = [xp.tile([P, NPG], F32, tag=f"x1T{dt}", name=f"x1T{dt}")
                       for dt in range(DT2)]
                for dt in range(DT2):
                    def drain_x1(ps, s0, w, dt=dt):
                        nc.vector.scalar_tensor_tensor(
                            out=x1T[dt][:, s0:s0 + w], in0=ps[:, 0:w],
                            scalar=bias_ap(l, 0, dt),
                            in1=hT_f[dt][:, s0:s0 + w],
                            op0=ALU.add, op1=ALU.add)
                    # wr@h terms first: they only need h, so the PE can start
                    # while the ReduceScatter readback is still landing
                    lhs = ([wrT[l][:, kt, dt * P:(dt + 1) * P] for kt in range(DT2)]
                           + [wlT[l][:, kt, dt * P:(dt + 1) * P] for kt in range(DT2)])
                    rhs = [hT_b[kt][:] for kt in range(DT2)] \
                        + [agg_sb[kt][:] for kt in range(DT2)]
                    mm_slices(lhs, rhs, NPG, drain_x1)

                # ---- BN stats for n1 (x1), then the joint AllGather ----
                for dt in range(DT2):
                    emit_stats(x1T, dt, dt * 2)
                cc_in = dram.tile([P, 8], F32, tag="r1in", name="r1in")
                cc_out = dram.tile([NCORES, P, 8], F32, tag="r1out", name="r1out",
                                   addr_space="Shared")
                nc.sync.dma_start(out=cc_in[:], in_=stats[:])
                nc.gpsimd.collective_compute(
                    "AllGather", ALU.bypass, replica_groups=RG,
                    ins=[cc_in[:].opt()], outs=[cc_out[:].opt()],
                )
                gsum = small.tile([P, NCORES, 8], F32, tag="gsum", name="gsum")
                nc.sync.dma_start(out=gsum[:],
                                  in_=cc_out[:].rearrange("r p s -> p r s"))
                nc.vector.tensor_add(out=gsum[:, 0:4, :], in0=gsum[:, 0:4, :],
                                     in1=gsum[:, 4:8, :])
                nc.vector.tensor_add(out=gsum[:, 0:2, :], in0=gsum[:, 0:2, :],
                                     in1=gsum[:, 2:4, :])
                nc.vector.tensor_add(out=gsum[:, 0, :], in0=gsum[:, 0, :],
                                     in1=gsum[:, 1, :])
                gm = small.tile([P, 8], F32, tag="gm", name="gm")
                nc.vector.tensor_scalar(out=gm[:], in0=gsum[:, 0, :],
                                        scalar1=1.0 / N, scalar2=None,
                                        op0=ALU.mult)

                # batched scale/shift for n1 (cols 0,1) and n2 (cols 2,3), per dt
                def gap(t, off, n, stride):
                    a = t[:]
                    return bass.AP(tensor=a.tensor, offset=a.offset + off,
                                   ap=[list(a.ap[0])] + [[stride, n]])
                m4, e4 = gap(gm, 0, 4, 2), gap(gm, 1, 4, 2)
                var4 = small.tile([P, 4], F32, tag="var4", name="var4")
                sc4 = small.tile([P, 4], F32, tag="sc4", name="sc4")
                t4 = small.tile([P, 4], F32, tag="t4", name="t4")
                nc.vector.tensor_tensor(out=var4[:], in0=m4, in1=m4, op=ALU.mult)
                nc.vector.tensor_tensor(out=var4[:], in0=e4, in1=var4[:],
                                        op=ALU.subtract)
                nc.scalar.activation(out=var4[:], in_=var4[:], func=AF.Sqrt,
                                     bias=eps_t[:])
                nc.vector.reciprocal(out=var4[:], in_=var4[:])
                # w/b for (n1,dt0),(n1,dt1),(n2,dt0),(n2,dt1): nrm idx 0,2 / 1,3
                nv = nrmv[l][:]
                w4 = bass.AP(tensor=nv.tensor, offset=nv.offset,
                             ap=[list(nv.ap[0])] + [[4, 2], [1, 2]])
                b4 = bass.AP(tensor=nv.tensor, offset=nv.offset + 2,
                             ap=[list(nv.ap[0])] + [[4, 2], [1, 2]])
                nc.vector.tensor_tensor(out=sc4[:], in0=var4[:],
                                        in1=w4, op=ALU.mult)
                nc.vector.tensor_tensor(out=t4[:], in0=m4, in1=sc4[:], op=ALU.mult)
                nc.vector.tensor_tensor(out=t4[:], in0=b4, in1=t4[:],
                                        op=ALU.subtract)
                tc2 = small.tile([P, 2], F32, tag="tc2", name="tc2")
                nc.vector.tensor_add(out=tc2[:], in0=t4[:, 0:2], in1=t4[:, 2:4])

                # ---- out = n1(x1) + n2(x2) ----
                for dt in range(DT2):
                    for s in range(2):
                        sl = slice(s * 512, (s + 1) * 512)
                        nc.scalar.activation(out=outf[dt][:, sl], in_=x1T[dt][:, sl],
                                             func=AF.Identity,
                                             scale=sc4[:, dt:dt + 1],
                                             bias=tc2[:, dt:dt + 1])
                        nc.vector.scalar_tensor_tensor(
                            out=out_b[dt][:, sl], in0=x2T[dt][:, sl],
                            scalar=sc4[:, 2 + dt:3 + dt], in1=outf[dt][:, sl],
                            op0=ALU.mult, op1=ALU.add)

                # ---- MLP residual ----
                relu1 = [qkp.tile([P, NPG], BF16, tag=f"relu1{ft}", name=f"relu1{ft}")
                         for ft in range(FT4)]
                for ft in range(FT4):
                    def drain_r(ps, s0, w, ft=ft):
                        nc.scalar.activation(out=relu1[ft][:, s0:s0 + w],
                                             in_=ps[:, 0:w], func=AF.Relu,
                                             bias=b1v[l][:, ft:ft + 1])
                    mm_slices(
                        [w1T[l][:, kt, ft * P:(ft + 1) * P] for kt in range(DT2)],
                        [out_b[kt][:] for kt in range(DT2)], NPG, drain_r)
                out2 = [xp.tile([P, NPG], F32, tag=f"out2{dt}", name=f"out2{dt}")
                        for dt in range(DT2)]
                stats3 = small.tile([P, 4], F32, tag="stats3", name="stats3")
                for dt in range(DT2):
                    def drain_m(ps, s0, w, dt=dt):
                        nc.vector.scalar_tensor_tensor(
                            out=out2[dt][:, s0:s0 + w], in0=ps[:, 0:w],
                            scalar=bias_ap(l, 4, dt),
                            in1=out_b[dt][:, s0:s0 + w],
                            op0=ALU.add, op1=ALU.add)
                    mm_slices(
                        [w2T[l][:, kt, dt * P:(dt + 1) * P] for kt in range(FT4)],
                        [relu1[kt][:] for kt in range(FT4)], NPG, drain_m)
                for dt in range(DT2):
                    scr = tmpf if dt else outf[0]
                    c = dt * 2
                    if dt == 0:
                        nc.scalar.activation(out=scr[:], in_=out2[dt][:],
                                             func=AF.Identity,
                                             accum_out=stats3[:, c:c + 1])
                        nc.scalar.activation(out=scr[:], in_=out2[dt][:],
                                             func=AF.Square,
                                             accum_out=stats3[:, c + 1:c + 2])
                    else:
                        nc.vector.scalar_tensor_tensor(
                            out=scr[:], in0=out2[dt][:], scalar=0.0,
                            in1=out2[dt][:], op0=ALU.mult, op1=ALU.add,
                            accum_out=stats3[:, c:c + 1])
                        nc.vector.scalar_tensor_tensor(
                            out=scr[:], in0=out2[dt][:], scalar=1.0,
                            in1=out2[dt][:], op0=ALU.mult, op1=ALU.mult,
                            accum_out=stats3[:, c + 1:c + 2])
                if l == L - 1:
                    # pooled mean pieces: sum of h hidden under the AllGather
                    hsum = small.tile([P, DT2], F32, tag="hsum", name="hsum")
                    rsums = small.tile([P, 4], F32, tag="rsums", name="rsums")
                    for dt in range(DT2):
                        nc.vector.tensor_reduce(out=hsum[:, dt:dt + 1],
                                                in_=hT_f[dt][:],
                                                axis=mybir.AxisListType.X,
                                                op=ALU.add)
                cc3_in = dram.tile([P, 4], F32, tag="r2in", name="r2in")
                cc3_out = dram.tile([NCORES, P, 4], F32, tag="r2out", name="r2out",
                                    addr_space="Shared")
                nc.sync.dma_start(out=cc3_in[:], in_=stats3[:])
                nc.gpsimd.collective_compute(
                    "AllGather", ALU.bypass, replica_groups=RG,
                    ins=[cc3_in[:].opt()], outs=[cc3_out[:].opt()],
                )
                gsum3 = small.tile([P, NCORES, 4], F32, tag="gsum3", name="gsum3")
                nc.sync.dma_start(out=gsum3[:],
                                  in_=cc3_out[:].rearrange("r p s -> p r s"))
                nc.vector.tensor_add(out=gsum3[:, 0:4, :], in0=gsum3[:, 0:4, :],
                                     in1=gsum3[:, 4:8, :])
                nc.vector.tensor_add(out=gsum3[:, 0:2, :], in0=gsum3[:, 0:2, :],
                                     in1=gsum3[:, 2:4, :])
                nc.vector.tensor_add(out=gsum3[:, 0, :], in0=gsum3[:, 0, :],
                                     in1=gsum3[:, 1, :])
                g3 = small.tile([P, 4], F32, tag="g3", name="g3")
                nc.vector.tensor_scalar(out=g3[:], in0=gsum3[:, 0, :],
                                        scalar1=1.0 / N, scalar2=None,
                                        op0=ALU.mult)
                # batched over dt: m3 = cols 0,2 ; e3 = cols 1,3
                m2_, e2_ = gap(g3, 0, 2, 2), gap(g3, 1, 2, 2)
                v2 = small.tile([P, 2], F32, tag="v2", name="v2")
                r2 = small.tile([P, 2], F32, tag="r2", name="r2")
                al2 = small.tile([P, 2], F32, tag="al2", name="al2")
                be2 = small.tile([P, 2], F32, tag="be2", name="be2")
                nc.vector.tensor_tensor(out=v2[:], in0=m2_, in1=m2_, op=ALU.mult)
                nc.vector.tensor_tensor(out=v2[:], in0=e2_, in1=v2[:],
                                        op=ALU.subtract)
                nc.scalar.activation(out=r2[:], in_=v2[:], func=AF.Sqrt,
                                     bias=eps_t[:])
                nc.vector.reciprocal(out=r2[:], in_=r2[:])
                w3_ = bass.AP(tensor=nv.tensor, offset=nv.offset + 4 * 2,
                              ap=[list(nv.ap[0])] + [[1, 2]])   # n3_w per dt
                bw_ = bass.AP(tensor=nv.tensor, offset=nv.offset + 6 * 2,
                              ap=[list(nv.ap[0])] + [[1, 2]])   # bn_w per dt
                bb_ = bass.AP(tensor=nv.tensor, offset=nv.offset + 7 * 2,
                              ap=[list(nv.ap[0])] + [[1, 2]])   # bn_b per dt
                # al = w3*r3; rbn = rsqrt(al^2*v3+eps); al = al*rbn*bw; be = bb-m3*al
                nc.vector.tensor_tensor(out=al2[:], in0=w3_, in1=r2[:], op=ALU.mult)
                nc.vector.tensor_tensor(out=be2[:], in0=al2[:], in1=al2[:],
                                        op=ALU.mult)
                nc.vector.tensor_tensor(out=be2[:], in0=be2[:], in1=v2[:],
                                        op=ALU.mult)
                nc.scalar.activation(out=be2[:], in_=be2[:], func=AF.Sqrt,
                                     bias=eps_t[:])
                nc.vector.reciprocal(out=be2[:], in_=be2[:])
                nc.vector.tensor_tensor(out=al2[:], in0=al2[:], in1=be2[:],
                                        op=ALU.mult)
                nc.vector.tensor_tensor(out=al2[:], in0=al2[:], in1=bw_, op=ALU.mult)
                nc.vector.tensor_tensor(out=be2[:], in0=m2_, in1=al2[:], op=ALU.mult)
                nc.vector.tensor_tensor(out=be2[:], in0=bb_, in1=be2[:],
                                        op=ALU.subtract)
                if l < L - 1:
                    hT_f_new = [featp.tile([P, NPG], F32, tag=f"hTf{dt}",
                                           name=f"hTf{dt}") for dt in range(DT2)]
                    hT_b_new = [featp.tile([P, NPG], BF16, tag=f"hTb{dt}",
                                           name=f"hTb{dt}") for dt in range(DT2)]
                    for dt in range(DT2):
                        for s in range(2):
                            sl = slice(s * 512, (s + 1) * 512)
                            nc.scalar.activation(out=tmpf[:, sl],
                                                 in_=out2[dt][:, sl],
                                                 func=AF.Relu,
                                                 scale=al2[:, dt:dt + 1],
                                                 bias=be2[:, dt:dt + 1])
                            nc.vector.tensor_add(out=hT_f_new[dt][:, sl],
                                                 in0=hT_f[dt][:, sl],
                                                 in1=tmpf[:, sl])
                        nc.gpsimd.tensor_copy(out=hT_b_new[dt][:],
                                              in_=hT_f_new[dt][:])
                    hT_f, hT_b = hT_f_new, hT_b_new
                else:
                    # last layer: h_new is only pooled, so accumulate the
                    # relu sums directly instead of materializing h_new
                    for dt in range(DT2):
                        for s in range(2):
                            sl = slice(s * 512, (s + 1) * 512)
                            nc.scalar.activation(out=tmpf[:, sl],
                                                 in_=out2[dt][:, sl],
                                                 func=AF.Relu,
                                                 scale=al2[:, dt:dt + 1],
                                                 bias=be2[:, dt:dt + 1],
                                                 accum_out=rsums[:, dt * 2 + s:
                                                                 dt * 2 + s + 1])

            # ---------------- pool + head ----------------
            pooled = small.tile([P, DT2], F32, tag="pooled", name="pooled")
            pooled_b = small.tile([P, DT2], BF16, tag="pooledb", name="pooledb")
            rp = rsums[:]
            nc.vector.tensor_tensor(
                out=pooled[:],
                in0=bass.AP(tensor=rp.tensor, offset=rp.offset,
                            ap=[list(rp.ap[0])] + [[2, 2]]),
                in1=bass.AP(tensor=rp.tensor, offset=rp.offset + 1,
                            ap=[list(rp.ap[0])] + [[2, 2]]),
                op=ALU.add)
            nc.vector.tensor_add(out=pooled[:], in0=pooled[:], in1=hsum[:])
            nc.scalar.activation(out=pooled_b[:], in_=pooled[:], func=AF.Identity,
                                 scale=1.0 / NPG)
            ps_y = psA.tile([P, 512], F32, space="PSUM", tag="a", name="a")
            for dt in range(DT2):
                nc.tensor.matmul(out=ps_y[0:OUT_D, 0:1],
                                 lhsT=w_outT[:, dt, :],
                                 rhs=pooled_b[:, dt:dt + 1],
                                 start=(dt == 0), stop=(dt == DT2 - 1))
            y_sb = small.tile([OUT_D, 1], F32, tag="ysb", name="ysb")
            nc.scalar.activation(out=y_sb[:], in_=ps_y[0:OUT_D, 0:1],
                                 func=AF.Identity, bias=boutv[:])
            nc.sync.dma_start(out=y_out[:, :], in_=y_sb[:])

    return nc


# ---------------------------------------------------------------------------
# Host-side: shard inputs, run, gather
# ---------------------------------------------------------------------------
def prep_inputs(x, edge_index, batch, w_in, b_in, sage_wl, sage_bl, sage_wr,
                attn_iw, attn_ib, attn_ow, attn_ob, n1_w, n1_b, n2_w, n2_b,
                n3_w, n3_b, mlp_w1, mlp_b1, mlp_w2, mlp_b2, bn_w, bn_b,
                w_out, b_out):
    bf = ml_dtypes.bfloat16
    f8 = ml_dtypes.float8_e4m3
    x = np.asarray(x, np.float32)
    ei = np.asarray(edge_index)
    src, dst = np.asarray(ei[0], np.int64), np.asarray(ei[1], np.int64)
    deg = np.bincount(dst, minlength=N).astype(np.float32)
    inv_deg = 1.0 / np.clip(deg, 1.0, None)

    def t32(a):
        return np.ascontiguousarray(np.asarray(a, np.float32))

    def packT(w_l):  # [out, in] -> [K=in/P, P, out] (transposed, packed)
        wt = t32(w_l).T  # [in, out]
        return wt.reshape(wt.shape[0] // P, P, wt.shape[1])

    shared = {
        "w_inT": t32(w_in).T.astype(bf),                       # [128, 256]
        "w_outT": packT(w_out).astype(bf),                     # [2, 128, 64]
        "wlT": np.stack([packT(sage_wl[l]) for l in range(L)]).astype(bf),
        "wrT": np.stack([packT(sage_wr[l]) for l in range(L)]).astype(bf),
        "wqT": np.stack([packT(attn_iw[l][0:D]) for l in range(L)]).astype(bf),
        "wkT": np.stack([packT(attn_iw[l][D:2 * D]) for l in range(L)]).astype(bf),
        "wvT": np.stack([packT(attn_iw[l][2 * D:3 * D]) for l in range(L)]).astype(bf),
        "owT": np.stack([packT(attn_ow[l]) for l in range(L)]).astype(bf),
        "w1T": np.stack([packT(mlp_w1[l]) for l in range(L)]).astype(bf),
        "w2T": np.stack([packT(mlp_w2[l]) for l in range(L)]).astype(bf),
        "vbr": np.stack([t32(attn_ib[l][2 * D:3 * D])[None, :]
                         for l in range(L)]).astype(bf),
        "b1v": np.stack([t32(mlp_b1[l]).reshape(FT4, P) for l in range(L)]),
        "boutv": t32(b_out)[:, None],
    }
    biasv = np.zeros((L, 8, DT2, P), np.float32)
    nrmv = np.zeros((L, 8, DT2, P), np.float32)
    for l in range(L):
        biasv[l, 0] = t32(sage_bl[l]).reshape(DT2, P)
        biasv[l, 1] = t32(attn_ib[l][0:D]).reshape(DT2, P)
        biasv[l, 2] = t32(attn_ib[l][D:2 * D]).reshape(DT2, P)
        biasv[l, 3] = t32(attn_ob[l]).reshape(DT2, P)
        biasv[l, 4] = t32(mlp_b2[l]).reshape(DT2, P)
        if l == 0:
            biasv[l, 5] = t32(b_in).reshape(DT2, P)
        nrmv[l, 0] = t32(n1_w[l]).reshape(DT2, P)
        nrmv[l, 1] = t32(n1_b[l]).reshape(DT2, P)
        nrmv[l, 2] = t32(n2_w[l]).reshape(DT2, P)
        nrmv[l, 3] = t32(n2_b[l]).reshape(DT2, P)
        nrmv[l, 4] = t32(n3_w[l]).reshape(DT2, P)
        nrmv[l, 5] = t32(n3_b[l]).reshape(DT2, P)
        nrmv[l, 6] = t32(bn_w[l]).reshape(DT2, P)
        nrmv[l, 7] = t32(bn_b[l]).reshape(DT2, P)
    shared["biasv"] = biasv
    shared["nrmv"] = nrmv

    in_maps = []
    for c in range(NCORES):
        lo, hi = c * NPG, (c + 1) * NPG
        sel = (src >= lo) & (src < hi)
        s_c, d_c = src[sel] - lo, dst[sel]
        at = np.zeros(NPG * N, np.float32)
        np.add.at(at, s_c * N + d_c, 1.0)
        m = dict(shared)
        m["xT"] = np.ascontiguousarray(x[lo:hi].T).astype(bf)
        m["at"] = at.reshape(NPG, N).astype(f8)
        m["invd"] = inv_deg[None, :].astype(bf)
        in_maps.append(m)
    return in_maps


_NC_CACHE = {}


def get_nc():
    if "nc" not in _NC_CACHE:
        _NC_CACHE["nc"] = build_kernel()
    return _NC_CACHE["nc"]


def kernel(**inputs):
    in_maps = prep_inputs(**inputs)
    nc = get_nc()
    res = run_bass_kernel_spmd(nc, in_maps, list(range(NCORES)))
    out = np.stack([res.results[c]["y"][:, 0] for c in range(NCORES)])
    return out.astype(np.float32)


if __name__ == "__main__":
    rng = np.random.default_rng(0)
    ins = dict(
        x=rng.standard_normal((N, IN_C), dtype=np.float32),
        edge_index=rng.integers(0, N, (2, E)),
        batch=np.arange(N, dtype=np.int32) // NPG,
        w_in=rng.standard_normal((D, IN_C), dtype=np.float32) * 0.05,
        b_in=rng.standard_normal(D, dtype=np.float32) * 0.05,
        sage_wl=rng.standard_normal((L, D, D), dtype=np.float32) * 0.05,
        sage_bl=rng.standard_normal((L, D), dtype=np.float32) * 0.05,
        sage_wr=rng.standard_normal((L, D, D), dtype=np.float32) * 0.05,
        attn_iw=rng.standard_normal((L, 3 * D, D), dtype=np.float32) * 0.05,
        attn_ib=rng.standard_normal((L, 3 * D), dtype=np.float32) * 0.05,
        attn_ow=rng.standard_normal((L, D, D), dtype=np.float32) * 0.05,
        attn_ob=rng.standard_normal((L, D), dtype=np.float32) * 0.05,
        n1_w=np.ones((L, D), np.float32), n1_b=np.zeros((L, D), np.float32),
        n2_w=np.ones((L, D), np.float32), n2_b=np.zeros((L, D), np.float32),
        n3_w=np.ones((L, D), np.float32), n3_b=np.zeros((L, D), np.float32),
        mlp_w1=rng.standard_normal((L, DFF, D), dtype=np.float32) * 0.05,
        mlp_b1=rng.standard_normal((L, DFF), dtype=np.float32) * 0.05,
        mlp_w2=rng.standard_normal((L, D, DFF), dtype=np.float32) * 0.05,
        mlp_b2=rng.standard_normal((L, D), dtype=np.float32) * 0.05,
        bn_w=np.ones((L, D), np.float32), bn_b=np.zeros((L, D), np.float32),
        w_out=rng.standard_normal((OUT_D, D), dtype=np.float32) * 0.05,
        b_out=rng.standard_normal(OUT_D, dtype=np.float32) * 0.05,
    )
    y = kernel(**ins)
    print("y shape:", y.shape, "finite:", np.isfinite(y).all())


# revision 56
# speedup vs baseline: 1.0737x; 1.0260x over previous
"""GPS (GraphGPS) forward pass on 8 Trainium2 NeuronCores.

Model (from the reference): 2 layers of
  SAGEConv(mean aggr) + residual + BN  ||  per-graph dense MHA + residual + BN
  -> sum branches -> MLP residual -> BN -> outer BN + relu + residual
then per-graph mean pool + linear head.

Sharding: one graph (1024 nodes) per core. The SAGE neighbor aggregation is
computed ReduceScatter-style: each core multiplies its LOCAL node features
h_c [1024, 256] against its src-slice of the dense edge-count matrix
A_c [1024 src x 8192 dst] (fp8 counts, exact small ints), producing partial
aggregates for ALL destinations; a ReduceScatter(add) then hands every core
the summed aggregate rows for its own 1024 destinations, which are scaled by
1/deg locally. This needs no AllGather of features at all. BatchNorm batch
stats are exchanged with small AllGathers (cheaper than AllReduce here) and
summed locally.

Device layout: features kept transposed (hT = [256 dims x 1024 nodes], dims
on partitions) so BN stats/apply are per-partition ops; h natural
([node, dim], from 16 PE transposes per layer) feeds the SAGE matmul as lhsT.
Attention: scores^T [keys, q] per (head, key-tile); exp on ACT; PV contracts
over keys with the 33-wide (V ++ ones) natural V so output lands natural
[q, d] with the softmax denominator on the same partition as its query row
(per-partition normalize), then 16 PE transposes take O back to d-major for
the out-projection.
"""
import numpy as np
import ml_dtypes

import concourse.bass as bass
import concourse.mybir as mybir
import concourse.tile as tile
from concourse.bass_utils import run_bass_kernel_spmd
from concourse.vector_clock import ScopedClock
from concourse.masks import make_identity

# ---------------------------------------------------------------------------
# Walrus workaround: this toolchain rejects >1 sync-wait command per
# instruction. Hoist excess waits onto same-engine NoOps / extra drains.
# ---------------------------------------------------------------------------
_MAX_WAITS = 1


def _split_waits_in_ordered(nc, ordered):
    for bb_name, insts in ordered.items():
        new_list = []
        for inst in insts:
            si = getattr(inst, "sync_info", None)
            if si is not None and si.on_wait and len(si.on_wait) > _MAX_WAITS:
                waits = list(si.on_wait)
                keep = waits[-_MAX_WAITS:]
                for w in waits[:-_MAX_WAITS]:
                    nop = mybir.InstNoOp(
                        name=nc.get_next_instruction_name(),
                        engine=inst.engine,
                        ins=[],
                        outs=[],
                        sync_info=mybir.SyncInfo(on_wait=[w], on_update=[]),
                    )
                    nop.debug = inst.debug
                    new_list.append(nop)
                si.on_wait[:] = keep
            new_list.append(inst)
        insts[:] = new_list


_orig_lower = tile.TileContext._lower_ordered_insts


def _patched_lower_ordered_insts(self, ordered):
    _split_waits_in_ordered(self.nc, ordered)
    return _orig_lower(self, ordered)


def _patched_drain_and_barrier(self, tick_clock, wait_clock):
    drain_inst = self.nc.sync.drain()
    wait_clock.add_sem_waits(drain_inst.ins, ScopedClock({None: tick_clock.global_clock}))
    si = drain_inst.ins.sync_info
    waits = list(si.on_wait) if si is not None else []
    if len(waits) > _MAX_WAITS:
        si.on_wait[:] = waits[:_MAX_WAITS]
        for w in waits[_MAX_WAITS:]:
            d2 = self.nc.sync.drain()
            d2.ins.sync_info = mybir.SyncInfo(on_wait=[w], on_update=[])
    self.nc.all_engine_barrier()
    assert self.sems is not None
    popped = self.nc._tile_sem_poison_stack.pop()
    assert popped is self._sem_poison
    self.nc.clear_and_free_semaphores(list(self.sems.allocated().values()))
    self.nc.all_engine_barrier()


tile.TileContext._lower_ordered_insts = _patched_lower_ordered_insts
tile.TileContext._drain_and_barrier = _patched_drain_and_barrier

# ---------------------------------------------------------------------------
# Problem constants (hardcoded per the task contract)
# ---------------------------------------------------------------------------
N, B, NPG = 8192, 8, 1024
D, H, DH, L = 256, 8, 32, 2
IN_C, OUT_D, E, DFF = 128, 64, 262144, 512
EPS = 1e-5
NCORES = 8
P = 128          # SBUF partitions
DT2 = D // P     # 2 dim tiles of 128
FT4 = DFF // P   # 4 ff tiles
NT8 = NPG // P   # 8 local node tiles
CH = 16          # dst chunks for the SAGE partial matmul
CHW = N // CH    # 512 dst per chunk
F32 = mybir.dt.float32
BF16 = mybir.dt.bfloat16
FP8 = mybir.dt.float8e4
AF = mybir.ActivationFunctionType
ALU = mybir.AluOpType
RG = [list(range(NCORES))]


def build_kernel():
    nc = bass.Bass()

    # ---- I/O declarations ----
    xT_in = nc.dram_tensor("xT", [P, NPG], BF16, kind="ExternalInput")
    at_in = nc.dram_tensor("at", [NPG, N], FP8, kind="ExternalInput")
    invd_in = nc.dram_tensor("invd", [1, N], BF16, kind="ExternalInput")
    # per-layer weights, host-transposed; leading dims packed for [128, ...] SBUF tiles
    wlT_in = nc.dram_tensor("wlT", [L, DT2, P, D], BF16, kind="ExternalInput")
    wrT_in = nc.dram_tensor("wrT", [L, DT2, P, D], BF16, kind="ExternalInput")
    wqT_in = nc.dram_tensor("wqT", [L, DT2, P, D], BF16, kind="ExternalInput")
    wkT_in = nc.dram_tensor("wkT", [L, DT2, P, D], BF16, kind="ExternalInput")
    wvT_in = nc.dram_tensor("wvT", [L, DT2, P, D], BF16, kind="ExternalInput")
    owT_in = nc.dram_tensor("owT", [L, DT2, P, D], BF16, kind="ExternalInput")
    w1T_in = nc.dram_tensor("w1T", [L, DT2, P, DFF], BF16, kind="ExternalInput")
    w2T_in = nc.dram_tensor("w2T", [L, FT4, P, D], BF16, kind="ExternalInput")
    w_inT_in = nc.dram_tensor("w_inT", [IN_C, D], BF16, kind="ExternalInput")
    w_outT_in = nc.dram_tensor("w_outT", [DT2, P, OUT_D], BF16, kind="ExternalInput")
    # biases / norm params, fp32; [idx, dt, p] so device holds [p, idx, dt]
    bias_in = nc.dram_tensor("biasv", [L, 8, DT2, P], F32, kind="ExternalInput")
    #   biasv[l]: 0=sage_b 1=qb 2=kb 3=ob 4=b2 5=b_in(l0) 6,7 spare
    b1_in = nc.dram_tensor("b1v", [L, FT4, P], F32, kind="ExternalInput")
    nrm_in = nc.dram_tensor("nrmv", [L, 8, DT2, P], F32, kind="ExternalInput")
    #   nrmv[l]: 0=n1_w 1=n1_b 2=n2_w 3=n2_b 4=n3_w 5=n3_b 6=bn_w 7=bn_b
    vb_in = nc.dram_tensor("vbr", [L, 1, D], BF16, kind="ExternalInput")
    bout_in = nc.dram_tensor("boutv", [OUT_D, 1], F32, kind="ExternalInput")

    y_out = nc.dram_tensor("y", [OUT_D, 1], F32, kind="ExternalOutput")

    with tile.TileContext(nc) as tc:
        with (
            tc.tile_pool(name="wpool", bufs=1) as wpool,      # persistent weights
            tc.tile_pool(name="featp", bufs=2) as featp,      # hT (old/new rotate)
            tc.tile_pool(name="natp", bufs=1) as natp,        # h natural + agg
            tc.tile_pool(name="qkp", bufs=1) as qkp,          # Q/K/V per layer
            tc.tile_pool(name="expp", bufs=16) as expp,       # exp(score) tiles
            tc.tile_pool(name="onp", bufs=1) as onp,          # O_nat / OT
            tc.tile_pool(name="xp", bufs=1) as xp,            # x1/x2/out/out2
            tc.tile_pool(name="stg", bufs=2) as stg,          # RS staging chunks
            tc.tile_pool(name="small", bufs=4) as small,      # stats etc
            tc.tile_pool(name="atp", bufs=4) as atp,          # A chunk stream
            tc.tile_pool(name="psA", bufs=2, space="PSUM") as psA,   # 2 banks
            tc.tile_pool(name="psS", bufs=2, space="PSUM") as psS,   # 4 banks
            tc.tile_pool(name="psV", bufs=1, space="PSUM") as psV,   # 1 bank
            tc.tile_pool(name="dram", bufs=2, space="DRAM") as dram,
        ):
            assert nc.vector.BN_STATS_FMAX >= 512

            # ---------------- load weights ----------------
            def load_w(shape, src_ap, name, dtype=BF16, pool=wpool):
                t = pool.tile(shape, dtype, tag=name, name=name)
                nc.sync.dma_start(out=t[:], in_=src_ap)
                return t

            # order matters: in_proj inputs + small params first so the first
            # matmuls aren't queued behind the big weight streams
            xT = load_w([P, NPG], xT_in[:, :], "xTw")
            w_inT = load_w([IN_C, D], w_inT_in[:, :], "w_inTw")
            biasv = [load_w([P, 8, DT2], bias_in[l].rearrange("i t p -> p i t"),
                            f"biasw{l}", F32) for l in range(L)]
            nrmv = [load_w([P, 8, DT2], nrm_in[l].rearrange("i t p -> p i t"),
                           f"nrmw{l}", F32) for l in range(L)]
            b1v = [load_w([P, FT4], b1_in[l].rearrange("t p -> p t"),
                          f"b1w{l}", F32) for l in range(L)]
            vbr = [load_w([1, D], vb_in[l], f"vbrw{l}") for l in range(L)]
            boutv = load_w([OUT_D, 1], bout_in[:, :], "boutw", F32)

            def load_packed(src, free, nm):
                # src [L, K, P, free] -> per-layer tiles [P, K, free]
                return [load_w([P, src.shape[1], free],
                               src[l].rearrange("k p f -> p k f"), f"{nm}{l}")
                        for l in range(L)]

            wqT = load_packed(wqT_in, D, "wqTw")
            wkT = load_packed(wkT_in, D, "wkTw")
            wvT = load_packed(wvT_in, D, "wvTw")
            owT = load_packed(owT_in, D, "owTw")
            wlT = load_packed(wlT_in, D, "wlTw")
            wrT = load_packed(wrT_in, D, "wrTw")
            w1T = load_packed(w1T_in, DFF, "w1Tw")
            w2T = load_packed(w2T_in, D, "w2Tw")
            w_outT = load_w([P, DT2, OUT_D], w_outT_in[:].rearrange("t p o -> p t o"),
                            "w_outTw")

            # global inv_deg, broadcast to all partitions: folded into the
            # SAGE partial drains (pre-ReduceScatter), so the RS result is
            # the finished mean aggregation
            invd_bc = wpool.tile([P, N], BF16, tag="invdbc", name="invdbc")
            iv_ap = invd_in[:, :]
            nc.sync.dma_start(
                out=invd_bc[:],
                in_=bass.AP(tensor=iv_ap.tensor, offset=iv_ap.offset,
                            ap=[[0, P]] + list(iv_ap.ap[1:])),
            )

            ones_row = wpool.tile([1, P], BF16)
            nc.vector.memset(ones_row[:], 1.0)
            eps_t = wpool.tile([P, 1], F32)
            nc.vector.memset(eps_t[:], EPS)
            ident = wpool.tile([P, P], F32)
            make_identity(nc, ident[:])

            def bias_ap(l, idx, dt):
                return biasv[l][:, idx, dt:dt + 1]

            def nrm_ap(l, idx, dt):
                return nrmv[l][:, idx, dt:dt + 1]

            # generic matmul into psA 512-slices with per-slice drain callback
            def mm_slices(lhsT_aps, rhs_aps, nfree, drain, slice_w=512):
                for s0 in range(0, nfree, slice_w):
                    w = min(slice_w, nfree - s0)
                    ps = psA.tile([P, 512], F32, space="PSUM", tag="a", name="a")
                    nk = len(lhsT_aps)
                    for k in range(nk):
                        nc.tensor.matmul(
                            out=ps[:, 0:w], lhsT=lhsT_aps[k],
                            rhs=rhs_aps[k][:, s0:s0 + w],
                            start=(k == 0), stop=(k == nk - 1),
                        )
                    drain(ps, s0, w)

            # ---------------- in_proj ----------------
            hT_f = [featp.tile([P, NPG], F32, tag=f"hTf{dt}", name=f"hTf{dt}")
                    for dt in range(DT2)]
            hT_b = [featp.tile([P, NPG], BF16, tag=f"hTb{dt}", name=f"hTb{dt}")
                    for dt in range(DT2)]
            for dt in range(DT2):
                def drain_in(ps, s0, w, dt=dt):
                    nc.scalar.activation(out=hT_f[dt][:, s0:s0 + w], in_=ps[:, 0:w],
                                         func=AF.Identity, bias=bias_ap(0, 5, dt))
                mm_slices([w_inT[:, dt * P:(dt + 1) * P]], [xT[:]], NPG, drain_in)
                nc.gpsimd.tensor_copy(out=hT_b[dt][:], in_=hT_f[dt][:])

            # ---------------- layers ----------------
            for l in range(L):
                # ---- A chunk prefetch (first 4; rest issued inside interleave)
                at_tiles = [None] * CH

                def fetch_chunk(c):
                    t = atp.tile([P, NT8, CHW], FP8, tag="att", name="att")
                    nc.sync.dma_start(
                        out=t[:],
                        in_=at_in[:, c * CHW:(c + 1) * CHW]
                        .rearrange("(kt p) f -> p kt f", p=P))
                    at_tiles[c] = t

                for c in range(4):
                    fetch_chunk(c)

                # ---- h natural via PE transposes (psV ring as scratch)
                # fp8 so the SAGE matmul can run in DoubleRow (2x) perf mode
                h_nat = natp.tile([P, NT8, D], FP8, tag="hnat", name="hnat")
                for nt in range(NT8):
                    for dt in range(DT2):
                        pst = psS.tile([P, NPG], F32, space="PSUM", tag="s", name="s")
                        nc.tensor.transpose(
                            out=pst[:, 0:P],
                            in_=hT_f[dt][:, nt * P:(nt + 1) * P],
                            identity=ident[:],
                        )
                        nc.vector.tensor_copy(out=h_nat[:, nt, dt * P:(dt + 1) * P],
                                              in_=pst[:, 0:P])

                # ---- Q/K projections (d-major) ----
                QT = [qkp.tile([P, NPG], BF16, tag=f"QT{dt}", name=f"QT{dt}")
                      for dt in range(DT2)]
                KT = [qkp.tile([P, NPG], BF16, tag=f"KT{dt}", name=f"KT{dt}")
                      for dt in range(DT2)]
                for dst, w_t, b_idx in ((QT, wqT[l], 1), (KT, wkT[l], 2)):
                    for dt in range(DT2):
                        def drain_qk(ps, s0, w, dst=dst, dt=dt, b_idx=b_idx):
                            nc.vector.tensor_scalar(
                                out=dst[dt][:, s0:s0 + w], in0=ps[:, 0:w],
                                scalar1=bias_ap(l, b_idx, dt), scalar2=None,
                                op0=ALU.add)
                        mm_slices(
                            [w_t[:, kt, dt * P:(dt + 1) * P] for kt in range(DT2)],
                            [hT_b[kt][:] for kt in range(DT2)], NPG, drain_qk)
                # stage head-3 rows (base partition 96 not addressable by PE lhsT)
                q_stg = [qkp.tile([DH, NPG], BF16, tag=f"qstg{dt}", name=f"qstg{dt}")
                         for dt in range(DT2)]
                k_stg = [qkp.tile([DH, NPG], BF16, tag=f"kstg{dt}", name=f"kstg{dt}")
                         for dt in range(DT2)]
                for dt in range(DT2):
                    nc.vector.tensor_copy(out=q_stg[dt][:], in_=QT[dt][96:128, :])
                    nc.vector.tensor_copy(out=k_stg[dt][:], in_=KT[dt][96:128, :])

                # ---- V natural per node tile with ones column (emitted inside
                # head-0's score slots, using the then-idle psV bank) ----
                Vn = [qkp.tile([P, H, DH + 1], BF16, tag=f"Vn{nt}", name=f"Vn{nt}")
                      for nt in range(NT8)]

                def emit_v(nt):
                    psv = psV.tile([P, 512], F32, space="PSUM", tag="v", name="v")
                    nc.tensor.matmul(out=psv[:, 0:D], lhsT=ones_row[:],
                                     rhs=vbr[l][:], start=True, stop=False)
                    for kt in range(DT2):
                        nc.tensor.matmul(
                            out=psv[:, 0:D],
                            lhsT=hT_b[kt][:, nt * P:(nt + 1) * P],
                            rhs=wvT[l][:, kt, :],
                            start=False, stop=(kt == DT2 - 1),
                        )
                    nc.vector.tensor_copy(out=Vn[nt][:, :, 0:DH], in_=psv[:, 0:D])
                    nc.vector.memset(Vn[nt][:, :, DH:DH + 1], 1.0)

                # ---- main interleave: attention scores/exp/PV + SAGE chunks ----
                scale = 1.0 / np.sqrt(DH)
                O_nat = onp.tile([P, NT8, D], F32, tag="onat", name="onat")
                agg_sb = [natp.tile([P, NPG], BF16, tag=f"aggsb{dt}",
                                    name=f"aggsb{dt}") for dt in range(DT2)]
                cc_rs_in = dram.tile([NCORES, DT2, P, NPG], BF16, tag="rsin",
                                     name="rsin")
                cc_rs_out = dram.tile([DT2, P, NPG], BF16, tag="rsout", name="rsout")

                # SAGE chunk emission state
                sage_state = {"next": 0, "mm": 0, "ps": None}

                def emit_sage_mms(n):
                    # emit up to n SAGE DoubleRow matmuls (kt pairs x dt)
                    for _ in range(n):
                        c = sage_state["next"]
                        if c >= CH:
                            return
                        if sage_state["mm"] == 0:
                            if at_tiles[c] is None:
                                fetch_chunk(c)
                            sage_state["ps"] = [
                                psA.tile([P, 512], F32, space="PSUM",
                                         tag="a", name="a")
                                for _ in range(DT2)]
                        i = sage_state["mm"]
                        j, dt = i // DT2, i % DT2
                        nc.tensor.matmul(
                            out=sage_state["ps"][dt][:],
                            lhsT=h_nat[:, 2 * j:2 * j + 2, dt * P:(dt + 1) * P],
                            rhs=at_tiles[c][:, 2 * j:2 * j + 2, :],
                            start=(j == 0), stop=(j == NT8 // 2 - 1),
                            perf_mode=mybir.MatmulPerfMode.DoubleRow,
                        )
                        sage_state["mm"] += 1
                        if sage_state["mm"] == NT8 // 2 * DT2:
                            # chunk complete: drain both dt planes + stage out
                            st = stg.tile([P, DT2, CHW], BF16, tag="stg", name="stg")
                            iv = invd_bc[:, c * CHW:(c + 1) * CHW]
                            nc.vector.tensor_tensor(out=st[:, 0, :], op=ALU.mult,
                                                    in0=sage_state["ps"][0][:], in1=iv)
                            nc.vector.tensor_tensor(out=st[:, 1, :], op=ALU.mult,
                                                    in0=sage_state["ps"][1][:], in1=iv)
                            cc, hh = c // 2, c % 2
                            # dispatch from the Pool queue: SP is busy with the
                            # A-chunk stream, and Pool owns the RS that waits
                            # on these writes anyway
                            nc.gpsimd.dma_start(
                                out=cc_rs_in[cc, :, :, hh * CHW:(hh + 1) * CHW]
                                .rearrange("t p f -> p t f"),
                                in_=st[:])
                            if c + 4 < CH:
                                fetch_chunk(c + 4)
                            sage_state["next"] = c + 1
                            sage_state["mm"] = 0

                def emit_pv_group(hp, exp_p, pv, qt):
                    for kt in range(NT8):
                        nc.tensor.matmul(
                            out=pv[:, qt * 64:qt * 64 + DH + 1],
                            lhsT=exp_p[kt][:, qt * P:(qt + 1) * P],
                            rhs=Vn[kt][:, hp, :],
                            start=(kt == 0), stop=(kt == NT8 - 1),
                        )

                def emit_pv_norm(hp, pv):
                    # batched reciprocal of the 8 denominators (col 32+64j)
                    pv_ap = pv[:]
                    den = bass.AP(tensor=pv_ap.tensor, offset=pv_ap.offset + DH,
                                  ap=[list(pv_ap.ap[0])] + [[64, NT8]])
                    rs_h = onp.tile([P, NT8], F32, tag=f"rs{hp % 2}",
                                    name=f"rs{hp % 2}")
                    nc.vector.reciprocal(out=rs_h[:], in_=den)
                    for qt in range(NT8):
                        nc.vector.tensor_scalar(
                            out=O_nat[:, qt, hp * DH:(hp + 1) * DH],
                            in0=pv[:, qt * 64:qt * 64 + DH],
                            scalar1=rs_h[:, qt:qt + 1], scalar2=None,
                            op0=ALU.mult)

                # PV of head h-1 is threaded through head h's score slots so
                # the PE never lumps 64 PV matmuls at a head boundary
                expt, expt_prev, pv_prev = {}, None, None
                for h in range(H):
                    qdt, qr = h // 4, DH * (h % 4)
                    q_src = QT[qdt] if qr < 96 else q_stg[qdt]
                    k_src = KT[qdt] if qr < 96 else k_stg[qdt]
                    qb_, qe_ = (qr, qr + DH) if qr < 96 else (0, DH)
                    for kt in range(NT8):
                        et = expp.tile([P, NPG], BF16, tag="expt", name="expt")
                        ps_sc = psS.tile([P, NPG], F32, space="PSUM",
                                         tag="s", name="s")
                        for s in range(2):
                            nc.tensor.matmul(
                                out=ps_sc[:, s * 512:(s + 1) * 512],
                                lhsT=k_src[qb_:qe_, kt * P:(kt + 1) * P],
                                rhs=q_src[qb_:qe_, s * 512:(s + 1) * 512],
                                start=True, stop=True,
                            )
                            emit_sage_mms(2)
                        nc.scalar.activation(out=et[:], in_=ps_sc[:],
                                             func=AF.Exp, scale=scale)
                        if h == 0:
                            emit_v(kt)
                        else:
                            emit_pv_group(h - 1, expt_prev, pv_prev, kt)
                            if kt == NT8 - 1:
                                emit_pv_norm(h - 1, pv_prev)
                        expt[kt] = et
                    expt_prev, expt = expt, {}
                    pv_prev = psV.tile([P, 512], F32, space="PSUM",
                                       tag="v", name="v")
                    emit_sage_mms(4)
                # drain the last head's PV
                for qt in range(NT8):
                    emit_pv_group(H - 1, expt_prev, pv_prev, qt)
                emit_pv_norm(H - 1, pv_prev)

                # ---- finish any remaining SAGE work, then ReduceScatter ----
                emit_sage_mms(CH * NT8 * DT2)
                nc.gpsimd.collective_compute(
                    "ReduceScatter", ALU.add, replica_groups=RG,
                    ins=[cc_rs_in[:].opt()], outs=[cc_rs_out[:].opt()],
                )
                nc.sync.dma_start(
                    out=agg_sb[0][:, 0:NPG], in_=cc_rs_out[0, :, :])
                nc.sync.dma_start(
                    out=agg_sb[1][:, 0:NPG], in_=cc_rs_out[1, :, :])

                # ---- O transposes to d-major + out projection -> x2 ----
                OT = [onp.tile([P, NPG], BF16, tag=f"OT{dt}", name=f"OT{dt}")
                      for dt in range(DT2)]
                for qt in range(NT8):
                    for dt in range(DT2):
                        pst = psS.tile([P, NPG], F32, space="PSUM", tag="s", name="s")
                        nc.tensor.transpose(
                            out=pst[:, 0:P],
                            in_=O_nat[:, qt, dt * P:(dt + 1) * P],
                            identity=ident[:],
                        )
                        nc.vector.tensor_copy(out=OT[dt][:, qt * P:(qt + 1) * P],
                                              in_=pst[:, 0:P])

                x2T = [xp.tile([P, NPG], F32, tag=f"x2T{dt}", name=f"x2T{dt}")
                       for dt in range(DT2)]
                for dt in range(DT2):
                    def drain_o(ps, s0, w, dt=dt):
                        nc.vector.scalar_tensor_tensor(
                            out=x2T[dt][:, s0:s0 + w], in0=ps[:, 0:w],
                            scalar=bias_ap(l, 3, dt),
                            in1=hT_f[dt][:, s0:s0 + w],
                            op0=ALU.add, op1=ALU.add)
                    mm_slices(
                        [owT[l][:, kt, dt * P:(dt + 1) * P] for kt in range(DT2)],
                        [OT[kt][:] for kt in range(DT2)], NPG, drain_o)

                # x2 stats up-front: x2 is ready before the RS result lands,
                # so these ops must precede the x1 drains in queue order.
                # Raw moments (sum x, sum x^2) via stt accumulators, dt0 on
                # gpsimd / dt1 on DVE so the two halves run in parallel.
                stats = small.tile([P, 8], F32, tag="stats", name="stats")
                outf = [xp.tile([P, NPG], F32, tag=f"outf{dt}", name=f"outf{dt}")
                        for dt in range(DT2)]
                out_b = [xp.tile([P, NPG], BF16, tag=f"outb{dt}", name=f"outb{dt}")
                         for dt in range(DT2)]
                tmpf = xp.tile([P, NPG], F32, tag="tmpf", name="tmpf")

                def emit_stats(xt, dt, c):
                    # raw moments; dt0 on ACT (Identity/Square are in every
                    # activation table - no table thrash), dt1 on DVE
                    scr = tmpf if dt else outf[0]
                    if dt == 0:
                        nc.scalar.activation(out=scr[:], in_=xt[dt][:],
                                             func=AF.Identity,
                                             accum_out=stats[:, c:c + 1])
                        nc.scalar.activation(out=scr[:], in_=xt[dt][:],
                                             func=AF.Square,
                                             accum_out=stats[:, c + 1:c + 2])
                    else:
                        nc.vector.scalar_tensor_tensor(
                            out=scr[:], in0=xt[dt][:], scalar=0.0, in1=xt[dt][:],
                            op0=ALU.mult, op1=ALU.add,
                            accum_out=stats[:, c:c + 1])
                        nc.vector.scalar_tensor_tensor(
                            out=scr[:], in0=xt[dt][:], scalar=1.0, in1=xt[dt][:],
                            op0=ALU.mult, op1=ALU.mult,
                            accum_out=stats[:, c + 1:c + 2])

                for dt in range(DT2):
                    emit_stats(x2T, dt, 4 + dt * 2)

                # ---- SAGE local transform -> x1 (needs RS result) ----
                x1T = [xp.tile([P, NPG], F32, tag=f"x1T{dt}", name=f"x1T{dt}")
                       for dt in range(DT2)]
                for dt in range(DT2):
                    def drain_x1(ps, s0, w, dt=dt):
                        nc.vector.scalar_tensor_tensor(
                            out=x1T[dt][:, s0:s0 + w], in0=ps[:, 0:w],
                            scalar=bias_ap(l, 0, dt),
                            in1=hT_f[dt][:, s0:s0 + w],
                            op0=ALU.add, op1=ALU.add)
                    # wr@h terms first: they only need h, so the PE can start
                    # while the ReduceScatter readback is still landing
                    lhs = ([wrT[l][:, kt, dt * P:(dt + 1) * P] for kt in range(DT2)]
                           + [wlT[l][:, kt, dt * P:(dt + 1) * P] for kt in range(DT2)])
                    rhs = [hT_b[kt][:] for kt in range(DT2)] \
                        + [agg_sb[kt][:] for kt in range(DT2)]
                    mm_slices(lhs, rhs, NPG, drain_x1)

                # ---- BN stats for n1 (x1), then the joint AllGather ----
                for dt in range(DT2):
                    emit_stats(x1T, dt, dt * 2)
                cc_in = dram.tile([P, 8], F32, tag="r1in", name="r1in")
                cc_out = dram.tile([NCORES, P, 8], F32, tag="r1out", name="r1out",
                                   addr_space="Shared")
                nc.sync.dma_start(out=cc_in[:], in_=stats[:])
                nc.gpsimd.collective_compute(
                    "AllGather", ALU.bypass, replica_groups=RG,
                    ins=[cc_in[:].opt()], outs=[cc_out[:].opt()],
                )
                gsum = small.tile([P, NCORES, 8], F32, tag="gsum", name="gsum")
                nc.sync.dma_start(out=gsum[:],
                                  in_=cc_out[:].rearrange("r p s -> p r s"))
                nc.vector.tensor_add(out=gsum[:, 0:4, :], in0=gsum[:, 0:4, :],
                                     in1=gsum[:, 4:8, :])
                nc.vector.tensor_add(out=gsum[:, 0:2, :], in0=gsum[:, 0:2, :],
                                     in1=gsum[:, 2:4, :])
                nc.vector.tensor_add(out=gsum[:, 0, :], in0=gsum[:, 0, :],
                                     in1=gsum[:, 1, :])
                gm = small.tile([P, 8], F32, tag="gm", name="gm")
                nc.vector.tensor_scalar(out=gm[:], in0=gsum[:, 0, :],
                                        scalar1=1.0 / N, scalar2=None,
                                        op0=ALU.mult)

                # batched scale/shift for n1 (cols 0,1) and n2 (cols 2,3), per dt
                def gap(t, off, n, stride):
                    a = t[:]
                    return bass.AP(tensor=a.tensor, offset=a.offset + off,
                                   ap=[list(a.ap[0])] + [[stride, n]])
                m4, e4 = gap(gm, 0, 4, 2), gap(gm, 1, 4, 2)
                var4 = small.tile([P, 4], F32, tag="var4", name="var4")
                sc4 = small.tile([P, 4], F32, tag="sc4", name="sc4")
                t4 = small.tile([P, 4], F32, tag="t4", name="t4")
                nc.vector.tensor_tensor(out=var4[:], in0=m4, in1=m4, op=ALU.mult)
                nc.vector.tensor_tensor(out=var4[:], in0=e4, in1=var4[:],
                                        op=ALU.subtract)
                nc.scalar.activation(out=var4[:], in_=var4[:], func=AF.Sqrt,
                                     bias=eps_t[:])
                nc.vector.reciprocal(out=var4[:], in_=var4[:])
                # w/b for (n1,dt0),(n1,dt1),(n2,dt0),(n2,dt1): nrm idx 0,2 / 1,3
                nv = nrmv[l][:]
                w4 = bass.AP(tensor=nv.tensor, offset=nv.offset,
                             ap=[list(nv.ap[0])] + [[4, 2], [1, 2]])
                b4 = bass.AP(tensor=nv.tensor, offset=nv.offset + 2,
                             ap=[list(nv.ap[0])] + [[4, 2], [1, 2]])
                nc.vector.tensor_tensor(out=sc4[:], in0=var4[:],
                                        in1=w4, op=ALU.mult)
                nc.vector.tensor_tensor(out=t4[:], in0=m4, in1=sc4[:], op=ALU.mult)
                nc.vector.tensor_tensor(out=t4[:], in0=b4, in1=t4[:],
                                        op=ALU.subtract)
                tc2 = small.tile([P, 2], F32, tag="tc2", name="tc2")
                nc.vector.tensor_add(out=tc2[:], in0=t4[:, 0:2], in1=t4[:, 2:4])

                # ---- out = n1(x1) + n2(x2) ----
                for dt in range(DT2):
                    for s in range(2):
                        sl = slice(s * 512, (s + 1) * 512)
                        nc.scalar.activation(out=outf[dt][:, sl], in_=x1T[dt][:, sl],
                                             func=AF.Identity,
                                             scale=sc4[:, dt:dt + 1],
                                             bias=tc2[:, dt:dt + 1])
                        nc.vector.scalar_tensor_tensor(
                            out=out_b[dt][:, sl], in0=x2T[dt][:, sl],
                            scalar=sc4[:, 2 + dt:3 + dt], in1=outf[dt][:, sl],
                            op0=ALU.mult, op1=ALU.add)

                # ---- MLP residual ----
                relu1 = [qkp.tile([P, NPG], BF16, tag=f"relu1{ft}", name=f"relu1{ft}")
                         for ft in range(FT4)]
                for ft in range(FT4):
                    def drain_r(ps, s0, w, ft=ft):
                        nc.scalar.activation(out=relu1[ft][:, s0:s0 + w],
                                             in_=ps[:, 0:w], func=AF.Relu,
                                             bias=b1v[l][:, ft:ft + 1])
                    mm_slices(
                        [w1T[l][:, kt, ft * P:(ft + 1) * P] for kt in range(DT2)],
                        [out_b[kt][:] for kt in range(DT2)], NPG, drain_r)
                out2 = [xp.tile([P, NPG], F32, tag=f"out2{dt}", name=f"out2{dt}")
                        for dt in range(DT2)]
                stats3 = small.tile([P, 4], F32, tag="stats3", name="stats3")
                for dt in range(DT2):
                    def drain_m(ps, s0, w, dt=dt):
                        nc.vector.scalar_tensor_tensor(
                            out=out2[dt][:, s0:s0 + w], in0=ps[:, 0:w],
                            scalar=bias_ap(l, 4, dt),
                            in1=out_b[dt][:, s0:s0 + w],
                            op0=ALU.add, op1=ALU.add)
                    mm_slices(
                        [w2T[l][:, kt, dt * P:(dt + 1) * P] for kt in range(FT4)],
                        [relu1[kt][:] for kt in range(FT4)], NPG, drain_m)
                for dt in range(DT2):
                    scr = tmpf if dt else outf[0]
                    c = dt * 2
                    if dt == 0:
                        nc.scalar.activation(out=scr[:], in_=out2[dt][:],
                                             func=AF.Identity,
                                             accum_out=stats3[:, c:c + 1])
                        nc.scalar.activation(out=scr[:], in_=out2[dt][:],
                                             func=AF.Square,
                                             accum_out=stats3[:, c + 1:c + 2])
                    else:
                        nc.vector.scalar_tensor_tensor(
                            out=scr[:], in0=out2[dt][:], scalar=0.0,
                            in1=out2[dt][:], op0=ALU.mult, op1=ALU.add,
                            accum_out=stats3[:, c:c + 1])
                        nc.vector.scalar_tensor_tensor(
                            out=scr[:], in0=out2[dt][:], scalar=1.0,
                            in1=out2[dt][:], op0=ALU.mult, op1=ALU.mult,
                            accum_out=stats3[:, c + 1:c + 2])
                if l == L - 1:
                    # pooled mean pieces: sum of h hidden under the AllGather
                    hsum = small.tile([P, DT2], F32, tag="hsum", name="hsum")
                    rsums = small.tile([P, 4], F32, tag="rsums", name="rsums")
                    for dt in range(DT2):
                        nc.vector.tensor_reduce(out=hsum[:, dt:dt + 1],
                                                in_=hT_f[dt][:],
                                                axis=mybir.AxisListType.X,
                                                op=ALU.add)
                cc3_in = dram.tile([P, 4], F32, tag="r2in", name="r2in")
                cc3_out = dram.tile([NCORES, P, 4], F32, tag="r2out", name="r2out",
                                    addr_space="Shared")
                nc.sync.dma_start(out=cc3_in[:], in_=stats3[:])
                nc.gpsimd.collective_compute(
                    "AllGather", ALU.bypass, replica_groups=RG,
                    ins=[cc3_in[:].opt()], outs=[cc3_out[:].opt()],
                )
                gsum3 = small.tile([P, NCORES, 4], F32, tag="gsum3", name="gsum3")
                nc.sync.dma_start(out=gsum3[:],
                                  in_=cc3_out[:].rearrange("r p s -> p r s"))
                nc.vector.tensor_add(out=gsum3[:, 0:4, :], in0=gsum3[:, 0:4, :],
                                     in1=gsum3[:, 4:8, :])
                nc.vector.tensor_add(out=gsum3[:, 0:2, :], in0=gsum3[:, 0:2, :],
                                     in1=gsum3[:, 2:4, :])
                nc.vector.tensor_add(out=gsum3[:, 0, :], in0=gsum3[:, 0, :],
                                     in1=gsum3[:, 1, :])
                g3 = small.tile([P, 4], F32, tag="g3", name="g3")
                nc.vector.tensor_scalar(out=g3[:], in0=gsum3[:, 0, :],
                                        scalar1=1.0 / N, scalar2=None,
                                        op0=ALU.mult)
                # batched over dt: m3 = cols 0,2 ; e3 = cols 1,3
                m2_, e2_ = gap(g3, 0, 2, 2), gap(g3, 1, 2, 2)
                v2 = small.tile([P, 2], F32, tag="v2", name="v2")
                r2 = small.tile([P, 2], F32, tag="r2", name="r2")
                al2 = small.tile([P, 2], F32, tag="al2", name="al2")
                be2 = small.tile([P, 2], F32, tag="be2", name="be2")
                nc.vector.tensor_tensor(out=v2[:], in0=m2_, in1=m2_, op=ALU.mult)
                nc.vector.tensor_tensor(out=v2[:], in0=e2_, in1=v2[:],
                                        op=ALU.subtract)
                nc.scalar.activation(out=r2[:], in_=v2[:], func=AF.Sqrt,
                                     bias=eps_t[:])
                nc.vector.reciprocal(out=r2[:], in_=r2[:])
                w3_ = bass.AP(tensor=nv.tensor, offset=nv.offset + 4 * 2,
                              ap=[list(nv.ap[0])] + [[1, 2]])   # n3_w per dt
                bw_ = bass.AP(tensor=nv.tensor, offset=nv.offset + 6 * 2,
                              ap=[list(nv.ap[0])] + [[1, 2]])   # bn_w per dt
                bb_ = bass.AP(tensor=nv.tensor, offset=nv.offset + 7 * 2,
                              ap=[list(nv.ap[0])] + [[1, 2]])   # bn_b per dt
                # al = w3*r3; rbn = rsqrt(al^2*v3+eps); al = al*rbn*bw; be = bb-m3*al
                nc.vector.tensor_tensor(out=al2[:], in0=w3_, in1=r2[:], op=ALU.mult)
                nc.vector.tensor_tensor(out=be2[:], in0=al2[:], in1=al2[:],
                                        op=ALU.mult)
                nc.vector.tensor_tensor(out=be2[:], in0=be2[:], in1=v2[:],
                                        op=ALU.mult)
                nc.scalar.activation(out=be2[:], in_=be2[:], func=AF.Sqrt,
                                     bias=eps_t[:])
                nc.vector.reciprocal(out=be2[:], in_=be2[:])
                nc.vector.tensor_tensor(out=al2[:], in0=al2[:], in1=be2[:],
                                        op=ALU.mult)
                nc.vector.tensor_tensor(out=al2[:], in0=al2[:], in1=bw_, op=ALU.mult)
                nc.vector.tensor_tensor(out=be2[:], in0=m2_, in1=al2[:], op=ALU.mult)
                nc.vector.tensor_tensor(out=be2[:], in0=bb_, in1=be2[:],
                                        op=ALU.subtract)
                if l < L - 1:
                    hT_f_new = [featp.tile([P, NPG], F32, tag=f"hTf{dt}",
                                           name=f"hTf{dt}") for dt in range(DT2)]
                    hT_b_new = [featp.tile([P, NPG], BF16, tag=f"hTb{dt}",
                                           name=f"hTb{dt}") for dt in range(DT2)]
                    for dt in range(DT2):
                        for s in range(2):
                            sl = slice(s * 512, (s + 1) * 512)
                            nc.scalar.activation(out=tmpf[:, sl],
                                                 in_=out2[dt][:, sl],
                                                 func=AF.Relu,
                                                 scale=al2[:, dt:dt + 1],
                                                 bias=be2[:, dt:dt + 1])
                            nc.vector.tensor_add(out=hT_f_new[dt][:, sl],
                                                 in0=hT_f[dt][:, sl],
                                                 in1=tmpf[:, sl])
                        nc.gpsimd.tensor_copy(out=hT_b_new[dt][:],
                                              in_=hT_f_new[dt][:])
                    hT_f, hT_b = hT_f_new, hT_b_new
                else:
                    # last layer: h_new is only pooled, so accumulate the
                    # relu sums directly instead of materializing h_new
                    for dt in range(DT2):
                        for s in range(2):
                            sl = slice(s * 512, (s + 1) * 512)
                            nc.scalar.activation(out=tmpf[:, sl],
                                                 in_=out2[dt][:, sl],
                                                 func=AF.Relu,
                                                 scale=al2[:, dt:dt + 1],
                                                 bias=be2[:, dt:dt + 1],
                                                 accum_out=rsums[:, dt * 2 + s:
                                                                 dt * 2 + s + 1])

            # ---------------- pool + head ----------------
            pooled = small.tile([P, DT2], F32, tag="pooled", name="pooled")
            pooled_b = small.tile([P, DT2], BF16, tag="pooledb", name="pooledb")
            rp = rsums[:]
            nc.vector.tensor_tensor(
                out=pooled[:],
                in0=bass.AP(tensor=rp.tensor, offset=rp.offset,
                            ap=[list(rp.ap[0])] + [[2, 2]]),
                in1=bass.AP(tensor=rp.tensor, offset=rp.offset + 1,
                            ap=[list(rp.ap[0])] + [[2, 2]]),
                op=ALU.add)
            nc.vector.tensor_add(out=pooled[:], in0=pooled[:], in1=hsum[:])
            nc.scalar.activation(out=pooled_b[:], in_=pooled[:], func=AF.Identity,
                                 scale=1.0 / NPG)
            ps_y = psA.tile([P, 512], F32, space="PSUM", tag="a", name="a")
            for dt in range(DT2):
                nc.tensor.matmul(out=ps_y[0:OUT_D, 0:1],
                                 lhsT=w_outT[:, dt, :],
                                 rhs=pooled_b[:, dt:dt + 1],
                                 start=(dt == 0), stop=(dt == DT2 - 1))
            y_sb = small.tile([OUT_D, 1], F32, tag="ysb", name="ysb")
            nc.scalar.activation(out=y_sb[:], in_=ps_y[0:OUT_D, 0:1],
                                 func=AF.Identity, bias=boutv[:])
            nc.sync.dma_start(out=y_out[:, :], in_=y_sb[:])

    return nc


# ---------------------------------------------------------------------------
# Host-side: shard inputs, run, gather
# ---------------------------------------------------------------------------
def prep_inputs(x, edge_index, batch, w_in, b_in, sage_wl, sage_bl, sage_wr,
                attn_iw, attn_ib, attn_ow, attn_ob, n1_w, n1_b, n2_w, n2_b,
                n3_w, n3_b, mlp_w1, mlp_b1, mlp_w2, mlp_b2, bn_w, bn_b,
                w_out, b_out):
    bf = ml_dtypes.bfloat16
    f8 = ml_dtypes.float8_e4m3
    x = np.asarray(x, np.float32)
    ei = np.asarray(edge_index)
    src, dst = np.asarray(ei[0], np.int64), np.asarray(ei[1], np.int64)
    deg = np.bincount(dst, minlength=N).astype(np.float32)
    inv_deg = 1.0 / np.clip(deg, 1.0, None)

    def t32(a):
        return np.ascontiguousarray(np.asarray(a, np.float32))

    def packT(w_l):  # [out, in] -> [K=in/P, P, out] (transposed, packed)
        wt = t32(w_l).T  # [in, out]
        return wt.reshape(wt.shape[0] // P, P, wt.shape[1])

    shared = {
        "w_inT": t32(w_in).T.astype(bf),                       # [128, 256]
        "w_outT": packT(w_out).astype(bf),                     # [2, 128, 64]
        "wlT": np.stack([packT(sage_wl[l]) for l in range(L)]).astype(bf),
        "wrT": np.stack([packT(sage_wr[l]) for l in range(L)]).astype(bf),
        "wqT": np.stack([packT(attn_iw[l][0:D]) for l in range(L)]).astype(bf),
        "wkT": np.stack([packT(attn_iw[l][D:2 * D]) for l in range(L)]).astype(bf),
        "wvT": np.stack([packT(attn_iw[l][2 * D:3 * D]) for l in range(L)]).astype(bf),
        "owT": np.stack([packT(attn_ow[l]) for l in range(L)]).astype(bf),
        "w1T": np.stack([packT(mlp_w1[l]) for l in range(L)]).astype(bf),
        "w2T": np.stack([packT(mlp_w2[l]) for l in range(L)]).astype(bf),
        "vbr": np.stack([t32(attn_ib[l][2 * D:3 * D])[None, :]
                         for l in range(L)]).astype(bf),
        "b1v": np.stack([t32(mlp_b1[l]).reshape(FT4, P) for l in range(L)]),
        "boutv": t32(b_out)[:, None],
    }
    biasv = np.zeros((L, 8, DT2, P), np.float32)
    nrmv = np.zeros((L, 8, DT2, P), np.float32)
    for l in range(L):
        biasv[l, 0] = t32(sage_bl[l]).reshape(DT2, P)
        biasv[l, 1] = t32(attn_ib[l][0:D]).reshape(DT2, P)
        biasv[l, 2] = t32(attn_ib[l][D:2 * D]).reshape(DT2, P)
        biasv[l, 3] = t32(attn_ob[l]).reshape(DT2, P)
        biasv[l, 4] = t32(mlp_b2[l]).reshape(DT2, P)
        if l == 0:
            biasv[l, 5] = t32(b_in).reshape(DT2, P)
        nrmv[l, 0] = t32(n1_w[l]).reshape(DT2, P)
        nrmv[l, 1] = t32(n1_b[l]).reshape(DT2, P)
        nrmv[l, 2] = t32(n2_w[l]).reshape(DT2, P)
        nrmv[l, 3] = t32(n2_b[l]).reshape(DT2, P)
        nrmv[l, 4] = t32(n3_w[l]).reshape(DT2, P)
        nrmv[l, 5] = t32(n3_b[l]).reshape(DT2, P)
        nrmv[l, 6] = t32(bn_w[l]).reshape(DT2, P)
        nrmv[l, 7] = t32(bn_b[l]).reshape(DT2, P)
    shared["biasv"] = biasv
    shared["nrmv"] = nrmv

    in_maps = []
    for c in range(NCORES):
        lo, hi = c * NPG, (c + 1) * NPG
        sel = (src >= lo) & (src < hi)
        s_c, d_c = src[sel] - lo, dst[sel]
        at = np.zeros(NPG * N, np.float32)
        np.add.at(at, s_c * N + d_c, 1.0)
        m = dict(shared)
        m["xT"] = np.ascontiguousarray(x[lo:hi].T).astype(bf)
        m["at"] = at.reshape(NPG, N).astype(f8)
        m["invd"] = inv_deg[None, :].astype(bf)
        in_maps.append(m)
    return in_maps


_NC_CACHE = {}


def get_nc():
    if "nc" not in _NC_CACHE:
        _NC_CACHE["nc"] = build_kernel()
    return _NC_CACHE["nc"]


def kernel(**inputs):
    in_maps = prep_inputs(**inputs)
    nc = get_nc()
    res = run_bass_kernel_spmd(nc, in_maps, list(range(NCORES)))
    out = np.stack([res.results[c]["y"][:, 0] for c in range(NCORES)])
    return out.astype(np.float32)


if __name__ == "__main__":
    rng = np.random.default_rng(0)
    ins = dict(
        x=rng.standard_normal((N, IN_C), dtype=np.float32),
        edge_index=rng.integers(0, N, (2, E)),
        batch=np.arange(N, dtype=np.int32) // NPG,
        w_in=rng.standard_normal((D, IN_C), dtype=np.float32) * 0.05,
        b_in=rng.standard_normal(D, dtype=np.float32) * 0.05,
        sage_wl=rng.standard_normal((L, D, D), dtype=np.float32) * 0.05,
        sage_bl=rng.standard_normal((L, D), dtype=np.float32) * 0.05,
        sage_wr=rng.standard_normal((L, D, D), dtype=np.float32) * 0.05,
        attn_iw=rng.standard_normal((L, 3 * D, D), dtype=np.float32) * 0.05,
        attn_ib=rng.standard_normal((L, 3 * D), dtype=np.float32) * 0.05,
        attn_ow=rng.standard_normal((L, D, D), dtype=np.float32) * 0.05,
        attn_ob=rng.standard_normal((L, D), dtype=np.float32) * 0.05,
        n1_w=np.ones((L, D), np.float32), n1_b=np.zeros((L, D), np.float32),
        n2_w=np.ones((L, D), np.float32), n2_b=np.zeros((L, D), np.float32),
        n3_w=np.ones((L, D), np.float32), n3_b=np.zeros((L, D), np.float32),
        mlp_w1=rng.standard_normal((L, DFF, D), dtype=np.float32) * 0.05,
        mlp_b1=rng.standard_normal((L, DFF), dtype=np.float32) * 0.05,
        mlp_w2=rng.standard_normal((L, D, DFF), dtype=np.float32) * 0.05,
        mlp_b2=rng.standard_normal((L, D), dtype=np.float32) * 0.05,
        bn_w=np.ones((L, D), np.float32), bn_b=np.zeros((L, D), np.float32),
        w_out=rng.standard_normal((OUT_D, D), dtype=np.float32) * 0.05,
        b_out=rng.standard_normal(OUT_D, dtype=np.float32) * 0.05,
    )
    y = kernel(**ins)
    print("y shape:", y.shape, "finite:", np.isfinite(y).all())
